# revision 65
# baseline (speedup 1.0000x reference)
"""Multi-head causal attention (B=2, T=2048, E=1024, H=16, D=64) on 8 trn2 cores.

Sharding: core c -> batch b = c // 4, head-group hg = c % 4 (4 heads each).
Per-core: QKV projections for its 4 heads, causal flash attention in
transposed-score layout (S^T[k,q]; softmax denominator folded into a
ones-augmented V matmul), row-parallel output projection producing a partial
[T, E] output. Host sums the 4 partials per batch and adds the bias.

v3: all attention matmuls in fp8 DoubleRow mode (0.5 cyc/col in the cost
model, vs 1.0 for bf16), with the DR 2-ktile layout satisfied without any
partition repacking:
 - S = K^T.Q: lhsT ktiles = (zeros-chunk, K-chunk) inside one kt tile via a
   step-sliced AP; rhs = Q broadcast stride-0 over the ktile dim (the zero
   weights make the duplicated-Q ktile contribute nothing). Q,K quantized to
   fp8e4m3 (3-term hi/lo projections kept for accuracy: qk errors dominate
   the error budget).
 - P.V: lhsT ktiles = (V_hi, V_lo) fp8 split (V exact to ~0.03%; V error
   enters the output linearly on small-neff rows, so it must stay tight);
   rhs = P (exp output written directly as fp8e4m3) broadcast stride-0.
Causal masking stays post-exp via tri-mask multiplies (DVE). Output
projection stays bf16. ACT (exp, ~58us of columns + per-instr overhead),
DVE (drain copies + normalize), and PE (~75us) end up balanced at ~76us
each; the schedule manages the in-order-queue hazards: proj drain bursts
are paced with a post-copy cooldown, y-proj units live in a separate
low-priority filler queue spaced ~3us apart in exp-time (their serial
matmul->PSUM-drain chains otherwise bunch where the proj queue empties
and head-of-line-block the PE at stream turns), prologue DMAs are
ordered pair-0-first (HWDGE issues serialize at ~650ns and transfers
serialize on the DMA pipe), and junk zero-matmuls at t=0 climb the PE
p-state ramp during the DMA wait.
"""
import collections
import sys
from contextlib import ExitStack

sys.path.insert(0, "/opt/trn_rl_repo")

import ml_dtypes
import numpy as np

import concourse.bass as bass
import concourse.tile as tile
from concourse import bacc, mybir
from concourse.bass_utils import run_bass_kernel_spmd

F32 = mybir.dt.float32
BF16 = mybir.dt.bfloat16
FP8 = mybir.dt.float8e4
DR = mybir.MatmulPerfMode.DoubleRow
EXP = mybir.ActivationFunctionType.Exp
SUB = mybir.AluOpType.subtract
MUL = mybir.AluOpType.mult

WSCALE = 32.0           # host prescale on Wq/Wk/Wv for fp8 range; q,k,v come
                        # out x32, folded into the exp scale and into Wp

B, T, E, H = 2, 2048, 1024, 16
D = E // H              # 64
N_CORES = 8
GH = 4                  # heads per core
GE = GH * D             # 256 per-core projection width
SCALE = float(D) ** -0.5

TCH = 512               # projection t-chunk
NTCH = T // TCH         # 4
KC = 8                  # contraction chunks of 128 over E
KC2 = KC // 2
QB = 512                # attention q-block
NQB = T // QB           # 4
KB = 128                # attention k-block
NKB = T // KB           # 16
VSLOT = 80              # v8 per-head slot width (64 data + ones col + pad, %16)

PE_NS = 1e9 / 2.4e9     # per moving-free column (bf16)
ACT_NS = 1e9 / 1.2e9    # per free column
EXP_OVH = 217.0         # ACT per-instruction overhead (access + decode)

DEFAULT_OPTS = dict(
    s_bufs=2,
    o_bufs=2,
    pv_bufs=2,
    p_bufs=7,
    x_bufs=4,
    l_bufs=6,
    on_bufs=8,
    yst_bufs=4,
    norm_splits=1,       # normalize split count (qb < last)
    norm_splits_last=4,  # normalize split count for the last q-block
    sem_lat=400.0,       # pacing fudge: SS-end -> exp-start latency
    ret_lat=1000.0,      # pacing fudge: exp-end -> O-start latency
    lead=0.0,            # pacing margin (ns)
    end_fill=1200.0,     # filler ns pulled at each stream end (norm window)
    copy_cd=1000.0,      # ns between a proj drain copy and the next slot alloc
    y_defer=1,           # 1: Y(qb) paced into phase qb+1; 3: all saved for last phase
    qk_copy_eng="vector",
    qk_terms=3,          # x*W terms for Q/K proj (1=hh only; 3=hh,lh,hl)
    tri_eng="vector",    # engine for the post-exp causal masking multiplies
    y_tail_split=True,   # alternate last-phase y drains between DVE and ACT
    early_qp0=False,     # drain next phase's qp0 at pair-1 start (DVE queue
                         # order: its copy lands before the phase-end bursts)
    sel_drain=True,      # drain() defers y units instead of emitting them
    cd_hist=1,           # which drain-copy (1=last) gates the next slot alloc
    early_act_copies=1,  # tch < this: qk/v drain copies go to ACT (idle early)
    warmup=80,           # junk DR matmuls at t=0 to climb the PE p-state ramp
    qk1_scalar=False,    # pair-1 q/k drain copies on ACT (idle at pair turn)
    y_si=4,              # y fillers allowed in streams si < y_si
    mid_qp1=False,       # drain this phase's qp1 mid-way through stream si1
    si_interleave=False, # stream order (0,0),(1,0),(0,1),(1,1) within a phase
    q_scalar=False,      # all q drain copies on ACT (they gate phase/pair turns)
    merge_heads=False,   # interleave both heads of a pair in one stream
    y_tail_wide=True,    # tail y units alternate pv/s psum pools (s is idle)
    pipe_depth=1,        # SS->OO software pipeline lag (groups)
    y_wide=False,        # alternate y psum pools in all phases, not just tail
    y_cd=3000.0,         # min act-clk ns between y filler pulls
    carry_oo=True,       # carry pending OOs across stream boundaries
    tri_in_ss=True,      # emit diag tri-muls right after the exp (not in OO)
    x_upfront=False,     # issue all x DMAs from the prologue (with x_bufs=8)
)


def build_program(opts=None):
    o = dict(DEFAULT_OPTS)
    if opts:
        o.update(opts)
    nc = bacc.Bacc("TRN2", target_bir_lowering=False, debug=False, num_devices=N_CORES)

    xh_d = nc.dram_tensor("xh", [E, T], FP8, kind="ExternalInput").ap()
    xl_d = nc.dram_tensor("xl", [E, T], FP8, kind="ExternalInput").ap()
    wqh_d = nc.dram_tensor("wqh", [E, GE], FP8, kind="ExternalInput").ap()
    wql_d = nc.dram_tensor("wql", [E, GE], FP8, kind="ExternalInput").ap()
    wkh_d = nc.dram_tensor("wkh", [E, GE], FP8, kind="ExternalInput").ap()
    wkl_d = nc.dram_tensor("wkl", [E, GE], FP8, kind="ExternalInput").ap()
    wvh_d = nc.dram_tensor("wvh", [E, GE], FP8, kind="ExternalInput").ap()
    wvl_d = nc.dram_tensor("wvl", [E, GE], FP8, kind="ExternalInput").ap()
    wpt_d = nc.dram_tensor("wpt", [GE, E], BF16, kind="ExternalInput").ap()
    tri_d = nc.dram_tensor("tri", [KB, KB], FP8, kind="ExternalInput").ap()
    y_d = nc.dram_tensor("y", [T, E], BF16, kind="ExternalOutput").ap()

    with tile.TileContext(nc) as tc:
        with tc.tile_pool(name="weights", bufs=1) as wpool, \
             tc.tile_pool(name="qk", bufs=1) as qkpool, \
             tc.tile_pool(name="vsb", bufs=1) as vpool, \
             tc.tile_pool(name="xin", bufs=o["x_bufs"]) as xpool, \
             tc.tile_pool(name="ptile", bufs=o["p_bufs"]) as ppool, \
             tc.tile_pool(name="lbc", bufs=o["l_bufs"]) as lpool, \
             tc.tile_pool(name="onorm", bufs=o["on_bufs"]) as onpool, \
             tc.tile_pool(name="ystage", bufs=o["yst_bufs"]) as ypool, \
             tc.tile_pool(name="s_ps", bufs=o["s_bufs"], space="PSUM") as s_ps, \
             tc.tile_pool(name="pv_ps", bufs=o["pv_bufs"], space="PSUM") as pv_ps, \
             tc.tile_pool(name="o_ps", bufs=o["o_bufs"], space="PSUM") as o_ps:
            qk_ps = v_ps = pv_ps

            nqk = 2 if o["qk_terms"] > 1 else 1
            wq_sb = [wpool.tile([128, KC2, 2, GE], FP8, name=f"wq{i}") for i in range(nqk)]
            wk_sb = [wpool.tile([128, KC2, 2, GE], FP8, name=f"wk{i}") for i in range(nqk)]
            wv_sb = [wpool.tile([128, KC2, 2, GE], FP8, name=f"wv{i}") for i in range(2)]
            wp_sb = wpool.tile([128, 2, E], BF16)
            tri_sb = wpool.tile([KB, KB], FP8)

            # Q^T per pair: [p=(h%2,d), pair, t] fp8
            qt_sb = qkpool.tile([128, 2, T], FP8)
            # K^T chunked: chunks 0 and 33 = zeros (DR ktile partner / warmup
            # operands), chunks 1+pair*16+j = K^T[:, j*128:(j+1)*128]
            kt_sb = qkpool.tile([128, 2 * NKB + 2, KB], FP8)
            # V: [p=key%128, tblock, hi/lo, head*80 + (d | ones at 64 | pad)]
            v_sb = vpool.tile([128, NKB, 2, GH * VSLOT], FP8)

            xts = [None] * NTCH  # per-tch ([hi, lo]) [128, KC2, 2, TCH] tiles

            def dr(ap3):
                # dram [rows, n] -> DoubleRow sbuf layout [p, c2, i, n]
                return ap3.rearrange("(c i p) n -> p c i n", i=2, p=128)

            def w_dma(w_sb_t, w_d):
                nc.sync.dma_start(out=w_sb_t[:], in_=dr(w_d))

            def emit_x_dma(tch):
                xts[tch] = [xpool.tile([128, KC2, 2, TCH], FP8, tag="xt",
                                       name=f"x{tch}_{hl}") for hl in range(2)]
                xsrc = [x_d[:, tch * TCH:(tch + 1) * TCH] for x_d in (xh_d, xl_d)]
                if tch == 0:
                    # prologue: HWDGE serializes DMA issue at ~650ns each and
                    # transfers serialize on the DMA pipe, so order by first
                    # use and load only the pair-0 weight columns up front:
                    # the pair-0 Q/K projection chain gates the first exp.
                    def w_half(w_sb_t, w_d, pair):
                        sl = slice(pair * 128, (pair + 1) * 128)
                        nc.sync.dma_start(out=w_sb_t[:, :, :, sl],
                                          in_=dr(w_d)[:, :, :, sl])
                    nc.sync.dma_start(out=xts[0][0][:], in_=dr(xsrc[0]))
                    w_half(wq_sb[0], wqh_d, 0)
                    if nqk > 1:
                        nc.sync.dma_start(out=xts[0][1][:], in_=dr(xsrc[1]))
                        w_half(wq_sb[1], wql_d, 0)
                    w_half(wk_sb[0], wkh_d, 0)
                    if nqk > 1:
                        w_half(wk_sb[1], wkl_d, 0)
                    w_dma(wv_sb[0], wvh_d)
                    w_dma(wv_sb[1], wvl_d)
                    if nqk == 1:
                        nc.sync.dma_start(out=xts[0][1][:], in_=dr(xsrc[1]))
                    w_half(wq_sb[0], wqh_d, 1)
                    if nqk > 1:
                        w_half(wq_sb[1], wql_d, 1)
                    w_half(wk_sb[0], wkh_d, 1)
                    if nqk > 1:
                        w_half(wk_sb[1], wkl_d, 1)
                    nc.sync.dma_start(out=tri_sb[:], in_=tri_d)
                    emit_x_dma(1)
                    nc.sync.dma_start(
                        out=wp_sb[:], in_=wpt_d.rearrange("(c p) n -> p c n", p=128))
                    if o["x_upfront"]:
                        # all x chunks issued from the prologue: needs enough
                        # x bufs that no DMA waits a tile free (the in-order
                        # SP queue would head-block the y output DMAs)
                        emit_x_dma(2)
                        emit_x_dma(3)
                else:
                    for hl in range(2):
                        nc.sync.dma_start(out=xts[tch][hl][:], in_=dr(xsrc[hl]))

            # ---- pacing state ------------------------------------------------
            clk = {"pe": 0.0, "act": 0.0, "allow_y": False, "last_y": -1e9}
            pend = collections.deque()  # carried (pt, oo, hook, after) entries

            def pop_oo():
                pt_, oo_, hook_, after_ = pend.popleft()
                if hook_ is not None:
                    hook_()
                oo_(pt_)
                if after_ is not None:
                    after_()
            copy_hist = collections.deque([-1e9] * 8, maxlen=8)
            fillers = collections.deque()    # proj units (tag, pe_ns, fn, allocs)
            fillers_y = collections.deque()  # y units: only emitted mid-stream

            def mm(pe_ns):
                clk["pe"] += pe_ns

            def emit_from(q):
                tag, pe_ns, fn, _alloc = q.popleft()
                marks = fn() or ()
                clk["pe"] += pe_ns
                if "copy" in marks:
                    copy_hist.append(clk["pe"])

            def emit_one():
                emit_from(fillers)

            def cd_blocked():
                # hold back a unit that re-allocates a shared proj psum slot
                # until the drain copy cd_hist groups back had time to land
                return clk["pe"] < copy_hist[-o["cd_hist"]] + o["copy_cd"]

            def pace(target):
                # proj fillers first; y units only mid-stream (a y matmul
                # stalls the in-order PE queue on its DVE drain copy, so they
                # must never sit ahead of a phase's first S matmuls) and
                # spaced out by act-clock so their copy chains hide under exps
                while clk["pe"] < target - o["lead"]:
                    if fillers and not (fillers[0][3] and cd_blocked()):
                        emit_from(fillers)
                    elif (fillers_y and clk["allow_y"]
                          and clk["act"] >= clk["last_y"] + o["y_cd"]):
                        clk["last_y"] = clk["act"]
                        emit_from(fillers_y)
                    else:
                        break

            def drain(tag_pred):
                while any(tag_pred(t) for t, _, _, _ in fillers):
                    emit_one()

            def qk_drain_copy(dst, src, tch=99, pair=0, is_q=False):
                if (o["qk_copy_eng"] == "scalar" or tch < o["early_act_copies"]
                        or (pair == 1 and o["qk1_scalar"])
                        or (is_q and o["q_scalar"])):
                    nc.scalar.copy(out=dst, in_=src)
                else:
                    nc.vector.tensor_copy(out=dst, in_=src)

            # ---- projection units -------------------------------------------
            TERMS = ((0, 0), (1, 0), (0, 1))  # (w hi/lo, x hi/lo): hh, lh, hl

            QK_TERMS = TERMS[:o["qk_terms"]]

            def qk_mms(ph, w_sb, tch, pair, c2):
                for ti, (wl, xl) in enumerate(QK_TERMS):
                    nc.tensor.matmul(
                        ph[:],
                        w_sb[wl][:, c2, :, pair * 128:(pair + 1) * 128],
                        xts[tch][xl][:, c2, :, :],
                        start=(c2 == 0 and ti == 0),
                        stop=(c2 == KC2 - 1 and ti == len(QK_TERMS) - 1),
                        perf_mode=DR)

            QKC = (TCH // 2) * PE_NS * o["qk_terms"]  # pe-ns per qk unit

            def q_units(tch):
                qp_h = {}

                def q_u(pair, c2):
                    def fn():
                        if c2 == 0:
                            qp_h[pair] = qk_ps.tile([128, TCH], F32, tag="pv",
                                                    name=f"q_{tch}_{pair}")
                        qk_mms(qp_h[pair], wq_sb, tch, pair, c2)
                        if c2 == KC2 - 1:
                            qk_drain_copy(
                                qt_sb[:, pair, tch * TCH:(tch + 1) * TCH],
                                qp_h[pair][:], tch, pair, is_q=True)
                            return ("copy",)
                    return fn
                return [(f"qp{pair}", QKC, q_u(pair, c2), c2 == 0)
                        for pair in range(2) for c2 in range(KC2)]

            def kv_units(tch):
                kp_h = {}

                def k_u(pair, c2):
                    def fn():
                        if c2 == 0:
                            kp_h[pair] = qk_ps.tile([128, TCH], F32, tag="pv",
                                                    name=f"k_{tch}_{pair}")
                        qk_mms(kp_h[pair], wk_sb, tch, pair, c2)
                        if c2 == KC2 - 1:
                            ch = 1 + pair * NKB + tch * (TCH // KB)
                            qk_drain_copy(
                                kt_sb[:, ch:ch + TCH // KB, :],
                                kp_h[pair].rearrange("p (c n) -> p c n", n=KB), tch, pair)
                            return ("copy",)
                    return fn

                units = [(f"kp{pair}", QKC, k_u(pair, c2), c2 == 0)
                         for pair in range(2) for c2 in range(KC2)]
                vp_h = {}

                def v_u(tsub, half):
                    def fn():
                        if half == 0:
                            vp_h[tsub] = v_ps.tile([128, GE], F32, tag="pv",
                                                   name=f"vp{tch}_{tsub}")
                        for c2 in range(2 * half, 2 * half + 2):
                            for ti, (wl, xl) in enumerate(TERMS):
                                nc.tensor.matmul(
                                    vp_h[tsub][:],
                                    xts[tch][xl][:, c2, :, tsub * KB:(tsub + 1) * KB],
                                    wv_sb[wl][:, c2, :, :],
                                    start=(c2 == 0 and ti == 0),
                                    stop=(c2 == KC2 - 1 and ti == len(TERMS) - 1),
                                    perf_mode=DR)
                        if half == 1:
                            tb = tch * (TCH // KB) + tsub
                            vsrc = vp_h[tsub].rearrange("p (h c) -> p h c", c=D)
                            vhi = v_sb.rearrange(
                                "p b i (h w) -> p b i h w", w=VSLOT)[:, tb, 0, :, 0:D]
                            vlo = v_sb.rearrange(
                                "p b i (h w) -> p b i h w", w=VSLOT)[:, tb, 1, :, 0:D]
                            if tch < o["early_act_copies"]:
                                nc.scalar.copy(out=vhi, in_=vsrc)
                            else:
                                nc.vector.tensor_copy(out=vhi, in_=vsrc)
                            nc.vector.tensor_tensor(out=vlo, in0=vsrc, in1=vhi, op=SUB)
                            return ("copy",)
                    return fn

                # shared proj psum slots: groups must stay contiguous
                k0 = [u for u in units if u[0] == "kp0"]
                k1 = [u for u in units if u[0] == "kp1"]
                vs = [("v", 3 * GE * PE_NS, v_u(t, half), half == 0)
                      for t in range(TCH // KB) for half in range(2)]
                return k0, vs, k1

            # ---- output-projection units ------------------------------------
            def y_units(qb, onorms):
                q0 = qb * QB
                units = []
                yt_h = {}
                late = qb >= NQB - 2  # runs in phase 3 / tail: pv pool is idle

                def y_unit(qt, nh):
                    def fn():
                        if nh == 0:
                            yt_h[qt] = ypool.tile([128, E], BF16, tag="yt", name=f"yt{qt}")
                        if (qb == NQB - 1 and o["y_tail_wide"]) or o["y_wide"]:
                            # y units are pulled as fillers when the proj queue
                            # is empty, so pv slots are free then: alternating
                            # pools doubles the slots and compresses the
                            # copy-wait ping-pong chain at stream turns
                            pool = pv_ps if (qt + nh) % 2 else s_ps
                            yp = pool.tile([128, 512], F32,
                                           tag="pv" if (qt + nh) % 2 else "s", name="yp")
                        elif late:
                            yp = pv_ps.tile([128, 512], F32, tag="pv", name="yp")
                        else:
                            yp = s_ps.tile([128, 512], F32, tag="s", name="yp")
                        for pair in range(2):
                            nc.tensor.matmul(yp[:],
                                             onorms[pair][:, qt * 128:(qt + 1) * 128],
                                             wp_sb[:, pair, nh * 512:(nh + 1) * 512],
                                             start=(pair == 0), stop=(pair == 1))
                        ysl = yt_h[qt][:, nh * 512:(nh + 1) * 512]
                        if qb == NQB - 1 and o["y_tail_split"] and (qt + nh) % 2:
                            nc.scalar.copy(out=ysl, in_=yp[:])
                        else:
                            nc.vector.tensor_copy(out=ysl, in_=yp[:])
                        nc.sync.dma_start(
                            out=y_d[q0 + qt * 128:q0 + (qt + 1) * 128, nh * 512:(nh + 1) * 512],
                            in_=ysl)
                    return fn

                for qt in range(QB // 128):
                    for nh in range(2):
                        units.append(("y", 2 * 512 * PE_NS, y_unit(qt, nh), False))
                return units

            # ---- attention stream -------------------------------------------
            def vslot(hb, hl):
                # [128, 2, 65] hi/lo ktile view of head hb at key-block j
                def at(j):
                    base = v_sb.rearrange("p b i (h w) -> p b i h w", w=VSLOT)
                    return base[:, j, :, hb, 0:D + 1]
                return at

            def kdr(pair, j, h):
                # lhsT [64, 2, 128]: ktile 0 = zeros chunk, ktile 1 = K chunk
                c = 1 + pair * NKB + j
                return kt_sb[h * D:(h + 1) * D, 0:c + 1:c, :]

            def bcast2(ap):
                return ap.unsqueeze(1).broadcast_to([ap.shape[0], 2, ap.shape[1]])

            def bcast2p(ap):
                return ap.unsqueeze(1).broadcast_to([128, 2, ap.shape[1]])

            def normalize(o_p, onorm, h, splits=1):
                w = QB // splits
                for s in range(splits):
                    qs = slice(s * w, (s + 1) * w)
                    strip = lpool.tile([1, w], F32, tag="strip")
                    nc.vector.reciprocal(out=strip[:], in_=o_p[D:D + 1, qs])
                    lb = lpool.tile([D, w], F32, tag="lb")
                    nc.gpsimd.partition_broadcast(lb[:], strip[:])
                    nc.vector.tensor_mul(onorm[h * D:(h + 1) * D, qs], o_p[0:D, qs], lb[:])

            def stream2(qb, pair, onorm, splits, prek=None, prev_v=None):
                # both heads of the pair interleaved in one software-pipelined
                # sequence: each exp is covered by the other head's matmuls and
                # the exp chain never breaks at a head boundary
                q0 = qb * QB
                nk = (q0 + QB) // KB
                nfull = nk - 4
                ngrp = nk // 2
                o_ps_h = [o_ps.tile([D + 1, QB], F32, tag="o", name=f"o2_{h}")
                          for h in range(2)]

                def grp2(h, j2, diag):
                    bsl = slice(h * D, h * D + D)
                    hb = pair * 2 + h
                    vat = vslot(hb, 0)
                    o_p = o_ps_h[h]
                    qrhs = qt_sb[bsl, pair, q0:q0 + QB]
                    r0 = (j2 - nfull) * KB if diag else 0
                    r1 = r0 + KB
                    w1 = QB - r1

                    def ss():
                        sp = s_ps.tile([128, 2 * QB], F32, tag="s", name="sp")
                        pt = ppool.tile([128, 2 * QB], FP8, tag="p", name="pt")
                        if diag:
                            nc.tensor.matmul(sp[:, r0:QB], kdr(pair, j2, h),
                                             bcast2(qrhs[:, r0:]),
                                             start=True, stop=True, perf_mode=DR)
                            nc.tensor.matmul(sp[:, QB:QB + w1], kdr(pair, j2 + 1, h),
                                             bcast2(qrhs[:, r1:]),
                                             start=True, stop=True, perf_mode=DR)
                            mm((QB - r0 + w1) * PE_NS / 2)
                            nc.scalar.activation(out=pt[:, r0:QB + w1], in_=sp[:, r0:QB + w1],
                                                 func=EXP, scale=SCALE / (WSCALE * WSCALE))
                            clk["act"] = max(clk["act"], clk["pe"] + o["sem_lat"])                                 + (QB - r0 + w1) * ACT_NS + EXP_OVH
                        else:
                            for jj in range(2):
                                nc.tensor.matmul(sp[:, jj * QB:(jj + 1) * QB],
                                                 kdr(pair, j2 + jj, h), bcast2(qrhs),
                                                 start=True, stop=True, perf_mode=DR)
                            mm(2 * QB * PE_NS / 2)
                            nc.scalar.activation(out=pt[:], in_=sp[:], func=EXP,
                                                 scale=SCALE / (WSCALE * WSCALE))
                            clk["act"] = max(clk["act"], clk["pe"] + o["sem_lat"])                                 + 2 * QB * ACT_NS + EXP_OVH
                        return pt

                    def oo(pt):
                        if diag:
                            if not o["tri_in_ss"]:
                                tri_e = nc.gpsimd if o["tri_eng"] == "pool" else nc.vector
                                tri_e.tensor_tensor(out=pt[:, r0:r0 + KB],
                                                    in0=pt[:, r0:r0 + KB], in1=tri_sb[:], op=MUL)
                                tri_e.tensor_tensor(out=pt[:, QB:QB + KB],
                                                    in0=pt[:, QB:QB + KB], in1=tri_sb[:], op=MUL)
                            nc.tensor.matmul(o_p[:, r0:QB], vat(j2),
                                             bcast2p(pt[:, r0:QB]),
                                             start=(j2 == 0), stop=False, perf_mode=DR)
                            nc.tensor.matmul(o_p[:, r1:QB], vat(j2 + 1),
                                             bcast2p(pt[:, QB:QB + w1]),
                                             start=False, stop=(j2 + 1 == nk - 1),
                                             perf_mode=DR)
                            mm((QB - r0 + w1) * PE_NS / 2)
                        else:
                            for jj in range(2):
                                j = j2 + jj
                                nc.tensor.matmul(o_p[:], vat(j),
                                                 bcast2p(pt[:, jj * QB:(jj + 1) * QB]),
                                                 start=(j == 0), stop=False, perf_mode=DR)
                            mm(2 * QB * PE_NS / 2)
                    return ss, oo

                seq = []
                for g in range(ngrp):
                    j2 = 2 * g
                    diag = j2 >= nfull
                    for h in range(2):
                        seq.append((h, g, grp2(h, j2, diag)))
                prev = None
                for idx, (h, g, (ss, oo)) in enumerate(seq):
                    if idx == max(0, len(seq) - 4) and prek is not None:
                        prek()
                    pt = ss()
                    pace(clk["act"] + o["ret_lat"])
                    if prev is not None:
                        if prev[3] == max(0, len(seq) - 4) and prev_v is not None:
                            prev_v()
                        prev[1](prev[0])
                        if prev[4] == ngrp - 1:
                            normalize(o_ps_h[prev[2]], onorm, prev[2], splits)
                    prev = (pt, oo, h, idx, g)
                if prev[3] == max(0, len(seq) - 4) and prev_v is not None:
                    prev_v()
                prev[1](prev[0])
                normalize(o_ps_h[prev[2]], onorm, prev[2], splits)
                pace(clk["pe"] + o["end_fill"])

            def stream(qb, pair, h, onorm, splits, prek=None, prev_v=None,
                       mid=None):
                q0 = qb * QB
                nk = (q0 + QB) // KB
                nfull = nk - 4
                bsl = slice(h * D, h * D + D)
                hb = pair * 2 + h
                vat = vslot(hb, 0)
                o_p = o_ps.tile([D + 1, QB], F32, tag="o")
                qrhs = qt_sb[bsl, pair, q0:q0 + QB]

                def grp(j2, diag):
                    r0 = (j2 - nfull) * KB if diag else 0
                    r1 = r0 + KB
                    w1 = QB - r1

                    def ss():
                        sp = s_ps.tile([128, 2 * QB], F32, tag="s", name="sp")
                        pt = ppool.tile([128, 2 * QB], FP8, tag="p", name="pt")
                        if diag:
                            nc.tensor.matmul(sp[:, r0:QB], kdr(pair, j2, h),
                                             bcast2(qrhs[:, r0:]),
                                             start=True, stop=True, perf_mode=DR)
                            nc.tensor.matmul(sp[:, QB:QB + w1], kdr(pair, j2 + 1, h),
                                             bcast2(qrhs[:, r1:]),
                                             start=True, stop=True, perf_mode=DR)
                            mm((QB - r0 + w1) * PE_NS / 2)
                            nc.scalar.activation(out=pt[:, r0:QB + w1], in_=sp[:, r0:QB + w1],
                                                 func=EXP, scale=SCALE / (WSCALE * WSCALE))
                            clk["act"] = max(clk["act"], clk["pe"] + o["sem_lat"]) \
                                + (QB - r0 + w1) * ACT_NS + EXP_OVH
                            if o["tri_in_ss"]:
                                tri_e = nc.gpsimd if o["tri_eng"] == "pool" else nc.vector
                                tri_e.tensor_tensor(out=pt[:, r0:r0 + KB],
                                                    in0=pt[:, r0:r0 + KB], in1=tri_sb[:], op=MUL)
                                tri_e.tensor_tensor(out=pt[:, QB:QB + KB],
                                                    in0=pt[:, QB:QB + KB], in1=tri_sb[:], op=MUL)
                        else:
                            for jj in range(2):
                                j = j2 + jj
                                nc.tensor.matmul(sp[:, jj * QB:(jj + 1) * QB],
                                                 kdr(pair, j, h), bcast2(qrhs),
                                                 start=True, stop=True, perf_mode=DR)
                            mm(2 * QB * PE_NS / 2)
                            nc.scalar.activation(out=pt[:], in_=sp[:], func=EXP,
                                                 scale=SCALE / (WSCALE * WSCALE))
                            clk["act"] = max(clk["act"], clk["pe"] + o["sem_lat"]) \
                                + 2 * QB * ACT_NS + EXP_OVH
                        return pt

                    def oo(pt):
                        if diag:
                            if not o["tri_in_ss"]:
                                tri_e = nc.gpsimd if o["tri_eng"] == "pool" else nc.vector
                                tri_e.tensor_tensor(out=pt[:, r0:r0 + KB],
                                                    in0=pt[:, r0:r0 + KB], in1=tri_sb[:], op=MUL)
                                tri_e.tensor_tensor(out=pt[:, QB:QB + KB],
                                                    in0=pt[:, QB:QB + KB], in1=tri_sb[:], op=MUL)
                            nc.tensor.matmul(o_p[:, r0:QB], vat(j2),
                                             bcast2p(pt[:, r0:QB]),
                                             start=(j2 == 0), stop=False, perf_mode=DR)
                            nc.tensor.matmul(o_p[:, r1:QB], vat(j2 + 1),
                                             bcast2p(pt[:, QB:QB + w1]),
                                             start=False, stop=(j2 + 1 == nk - 1),
                                             perf_mode=DR)
                            mm((QB - r0 + w1) * PE_NS / 2)
                        else:
                            for jj in range(2):
                                j = j2 + jj
                                nc.tensor.matmul(o_p[:], vat(j),
                                                 bcast2p(pt[:, jj * QB:(jj + 1) * QB]),
                                                 start=(j == 0), stop=False, perf_mode=DR)
                            mm(2 * QB * PE_NS / 2)
                    return ss, oo

                groups = [grp(j2, False) for j2 in range(0, nfull, 2)] \
                    + [grp(j2, True) for j2 in range(nfull, nk, 2)]
                # software-pipelined: SS/exp of groups g+1..g+depth run before
                # OO of g so the next S matmuls (plus fillers) cover the exp
                # latency; the queue is GLOBAL, so a stream's last OO(s) and
                # its normalize are emitted after the NEXT stream's first S
                # matmuls -- the ACT queue never drains at a stream turn
                ndiag = len(groups) - 2  # first group whose SS reads this tch's K
                depth = o["pipe_depth"]
                for gi, (ss, oo) in enumerate(groups):
                    if gi == min(1, ndiag) and mid is not None:
                        mid()
                    if gi == ndiag and prek is not None:
                        prek()
                    pt = ss()
                    pace(clk["act"] + o["ret_lat"])
                    if len(pend) >= depth:
                        pop_oo()
                    hook = prev_v if gi == ndiag else None
                    after = None
                    if gi == len(groups) - 1:
                        def after(o_p=o_p, onorm=onorm, h=h, splits=splits):
                            normalize(o_p, onorm, h, splits)
                    pend.append((pt, oo, hook, after))
                if not o["carry_oo"]:
                    while pend:
                        pop_oo()
                pace(clk["pe"] + o["end_fill"])

            # ---- main schedule ----------------------------------------------
            nc.vector.memset(kt_sb[:, 0, :], 0.0)
            nc.vector.memset(kt_sb[:, 2 * NKB + 1, :], 0.0)
            if o["warmup"]:
                # climb the PE p-state ramp on junk zero matmuls while the
                # prologue DMAs land; results are discarded
                wu_ps = s_ps.tile([128, 2 * QB], F32, tag="s", name="wu")
                wu_lhs = kt_sb[0:64, 0::(2 * NKB + 1), :]
                wu_rhs = kt_sb[0:64, 0, :].unsqueeze(1).broadcast_to([64, 2, KB])
                for _ in range(o["warmup"]):
                    nc.tensor.matmul(wu_ps[:, 0:KB], wu_lhs, wu_rhs,
                                     start=True, stop=True, perf_mode=DR)
            emit_x_dma(0)
            vv = v_sb.rearrange("p b i (h w) -> p b i h w", w=VSLOT)
            nc.vector.memset(vv[:, :, 0, :, D:D + 1], 1.0)
            nc.vector.memset(vv[:, :, 1, :, D:D + 1], 0.0)

            def queue_proj(tch):
                qs = q_units(tch)
                q0 = [u for u in qs if u[0] == "qp0"]
                q1 = [u for u in qs if u[0] == "qp1"]
                k0, vs, k1 = kv_units(tch)
                for t, c, f, a in q0 + k0 + vs + q1 + k1:
                    fillers.append((f"{t}@{tch}", c, f, a))

            pending_y = []
            for qb in range(NQB):
                if qb == 0:
                    queue_proj(0)
                if 0 < qb < NQB - 1 and not o["x_upfront"]:
                    emit_x_dma(qb + 1)
                # this phase's Q (and for streams' diagonals, K/V) must be
                # emitted before the attention that reads it
                drain(lambda t: t == f"qp0@{qb}")
                if qb < NQB - 1:
                    queue_proj(qb + 1)
                onorms = [onpool.tile([128, QB], BF16, tag="on", name=f"on{qb}_{i}") for i in range(2)]
                splits = o["norm_splits_last"] if qb == NQB - 1 else o["norm_splits"]
                if o["merge_heads"]:
                    for si, pair in enumerate((0, 1)):
                        clk["allow_y"] = si < 1 or qb == NQB - 1
                        if pair == 1:
                            drain(lambda t: t == f"qp1@{qb}")
                            if o["early_qp0"] and qb < NQB - 1:
                                drain(lambda t: t == f"qp0@{qb + 1}")
                        prek = lambda p=pair: drain(lambda t: t == f"kp{p}@{qb}")
                        prev_v = (lambda: drain(
                            lambda t: t == f"v@{qb}")) if pair == 0 else None
                        stream2(qb, pair, onorms[pair], splits, prek=prek,
                                prev_v=prev_v)
                        if si == 0 and pending_y and (
                                qb == NQB - 1 or o["y_defer"] == 1):
                            fillers_y.extend(pending_y)
                            pending_y = []
                    pending_y = pending_y + y_units(qb, onorms)
                    continue
                si_order = ((0, 0), (1, 0), (0, 1), (1, 1)) if o["si_interleave"] \
                    else ((0, 0), (0, 1), (1, 0), (1, 1))
                for si, (pair, h) in enumerate(si_order):
                    clk["allow_y"] = si < o["y_si"] or qb == NQB - 1
                    if pair == 1 and h == 0:
                        drain(lambda t: t == f"qp1@{qb}")
                        if o["early_qp0"] and qb < NQB - 1 and not o["si_interleave"]:
                            drain(lambda t: t == f"qp0@{qb + 1}")
                    if o["si_interleave"] and si == 2 and o["early_qp0"] and qb < NQB - 1:
                        drain(lambda t: t == f"qp0@{qb + 1}")
                    prek = (lambda p=pair: drain(
                        lambda t: t == f"kp{p}@{qb}")) if h == 0 else None
                    prev_v = (lambda: drain(
                        lambda t: t == f"v@{qb}")) if (pair == 0 and h == 0) else None
                    def mid_fn(q=qb):
                        drain(lambda t: t == f"qp1@{q}")
                        if o["early_qp0"] and q < NQB - 1:
                            drain(lambda t: t == f"qp0@{q + 1}")
                    mid = mid_fn if (o["mid_qp1"] and si == 1) else None
                    stream(qb, pair, h, onorms[pair], splits, prek=prek, prev_v=prev_v,
                           mid=mid)
                    if si == 0 and pending_y and (
                            qb == NQB - 1 or o["y_defer"] == 1):
                        fillers_y.extend(pending_y)
                        pending_y = []
                pending_y = pending_y + y_units(qb, onorms)
            while pend:
                pop_oo()
            clk["allow_y"] = True
            while fillers or fillers_y:
                emit_from(fillers if fillers else fillers_y)
            for _, _, fn, _a in pending_y:
                fn()

    nc.compile()
    return nc


_NC = {}


def _get_program(opts=None):
    key = tuple(sorted((opts or {}).items()))
    if key not in _NC:
        _NC[key] = build_program(opts)
    return _NC[key]


def _make_in_maps(x, Wq, Wk, Wv, Wp):
    bf = ml_dtypes.bfloat16
    f8 = ml_dtypes.float8_e4m3
    x32 = np.asarray(x, np.float32)
    xh = x32.astype(f8)
    xl = (x32 - xh.astype(np.float32)).astype(f8)

    def wsplit(W):
        wt = np.asarray(W, np.float32).T * WSCALE
        hi = wt.astype(f8)
        lo = (wt - hi.astype(np.float32)).astype(f8)
        return hi, lo

    wqh, wql = wsplit(Wq)
    wkh, wkl = wsplit(Wk)
    wvh, wvl = wsplit(Wv)
    wpt = np.ascontiguousarray((np.asarray(Wp, np.float32).T / WSCALE).astype(bf))
    tri = (np.arange(KB)[:, None] <= np.arange(KB)[None, :]).astype(f8)
    in_maps = []
    for c in range(N_CORES):
        b, hg = c // 4, c % 4
        gsl = slice(hg * GE, (hg + 1) * GE)
        in_maps.append({
            "xh": np.ascontiguousarray(xh[b].T),
            "xl": np.ascontiguousarray(xl[b].T),
            "wqh": np.ascontiguousarray(wqh[:, gsl]),
            "wql": np.ascontiguousarray(wql[:, gsl]),
            "wkh": np.ascontiguousarray(wkh[:, gsl]),
            "wkl": np.ascontiguousarray(wkl[:, gsl]),
            "wvh": np.ascontiguousarray(wvh[:, gsl]),
            "wvl": np.ascontiguousarray(wvl[:, gsl]),
            "wpt": np.ascontiguousarray(wpt[gsl, :]),
            "tri": tri,
        })
    return in_maps


def run_cores(x, Wq, Wk, Wv, Wp, bp, **spmd_kwargs):
    """Run the 8-core program; returns (y_full, BassKernelResults)."""
    nc = _get_program()
    in_maps = _make_in_maps(x, Wq, Wk, Wv, Wp)
    res = run_bass_kernel_spmd(nc, in_maps, list(range(N_CORES)), **spmd_kwargs)
    parts = [res.results[c]["y"] for c in range(N_CORES)]
    y = np.empty((B, T, E), np.float32)
    for b in range(B):
        acc = parts[4 * b].astype(np.float32)
        for hg in range(1, 4):
            acc = acc + parts[4 * b + hg].astype(np.float32)
        y[b] = acc
    y += np.asarray(bp, np.float32)[None, None, :]
    return y, res


def kernel(x, Wq, Wk, Wv, Wp, bp):
    y, _ = run_cores(x, Wq, Wk, Wv, Wp, bp)
    return y


def bench(x, Wq, Wk, Wv, Wp, bp, iters=12):
    """Time repeated on-device executions of the compiled program."""
    import time

    import jax
    import numpy as np_
    from jax.experimental.shard_map import shard_map
    from jax.sharding import Mesh, NamedSharding, PartitionSpec

    from concourse import bass2jax, mybir as mb

    nc = _get_program()
    in_maps = _make_in_maps(x, Wq, Wk, Wv, Wp)
    n_cores = N_CORES
    bass2jax.install_neuronx_cc_hook()

    partition_name = nc.partition_id_tensor.name if nc.partition_id_tensor else None
    in_names, out_names, out_avals, zero_outs = [], [], [], []
    for alloc in nc.m.functions[0].allocations:
        if not isinstance(alloc, mb.MemoryLocationSet):
            continue
        name = alloc.memorylocations[0].name
        if alloc.kind == "ExternalInput":
            if name != partition_name:
                in_names.append(name)
        elif alloc.kind == "ExternalOutput":
            out_names.append(name)
            shape = tuple(alloc.tensor_shape)
            dtype = mb.dt.np(alloc.dtype)
            out_avals.append(jax.core.ShapedArray(shape, dtype))
            zero_outs.append(np_.zeros(shape, dtype))
    n_params = len(in_names)
    all_in_names = in_names + out_names
    if partition_name is not None:
        all_in_names = all_in_names + [partition_name]

    def _body(*args):
        operands = list(args)
        if partition_name is not None:
            operands.append(bass2jax.partition_id_tensor())
        outs = bass2jax._bass_exec_p.bind(
            *operands,
            out_avals=tuple(out_avals),
            in_names=tuple(all_in_names),
            out_names=tuple(out_names),
            lowering_input_output_aliases=(),
            sim_require_finite=True,
            sim_require_nnan=True,
            nc=nc,
        )
        return tuple(outs)

    devices = jax.devices()[:n_cores]
    mesh = Mesh(np_.asarray(devices), ("core",))
    donate = tuple(range(n_params, n_params + len(out_names)))
    sharded = jax.jit(
        shard_map(_body, mesh=mesh,
                  in_specs=(PartitionSpec("core"),) * (n_params + len(out_names)),
                  out_specs=(PartitionSpec("core"),) * len(out_names),
                  check_rep=False),
        donate_argnums=donate, keep_unused=True,
    )
    sh = NamedSharding(mesh, PartitionSpec("core"))
    concat_in = [
        jax.device_put(
            np_.concatenate([np_.asarray(in_maps[c][nm]) for c in range(n_cores)], axis=0), sh)
        for nm in in_names
    ]
    zeros_np = [np_.zeros((n_cores * z.shape[0], *z.shape[1:]), z.dtype) for z in zero_outs]

    times = []
    out_arrs = None
    for it in range(iters):
        dz = [jax.device_put(z, sh) for z in zeros_np]
        jax.block_until_ready(dz)
        t0 = time.perf_counter()
        out_arrs = sharded(*concat_in, *dz)
        jax.block_until_ready(out_arrs)
        times.append(time.perf_counter() - t0)

    parts = [
        np_.asarray(out_arrs[i]).reshape(n_cores, *out_avals[i].shape)
        for i, nm in enumerate(out_names)
    ]
    yi = out_names.index("y")
    y = np_.empty((B, T, E), np_.float32)
    for b in range(B):
        acc = parts[yi][4 * b].astype(np_.float32)
        for hg in range(1, 4):
            acc = acc + parts[yi][4 * b + hg].astype(np_.float32)
        y[b] = acc
    y += np_.asarray(bp, np_.float32)[None, None, :]
    return y, times


# revision 68
# speedup vs baseline: 1.0851x; 1.0851x over previous
"""Multi-head causal attention (B=2, T=2048, E=1024, H=16, D=64) on 8 trn2 cores.

Sharding: core c -> batch b = c // 4, head-group hg = c % 4 (4 heads each).
Per-core: QKV projections for its 4 heads, causal flash attention in
transposed-score layout (S^T[k,q]; softmax denominator folded into a
ones-augmented V matmul), row-parallel output projection producing a partial
[T, E] output. Host sums the 4 partials per batch and adds the bias.

v3: all attention matmuls in fp8 DoubleRow mode (0.5 cyc/col in the cost
model, vs 1.0 for bf16), with the DR 2-ktile layout satisfied without any
partition repacking:
 - S = K^T.Q: lhsT ktiles = (zeros-chunk, K-chunk) inside one kt tile via a
   step-sliced AP; rhs = Q broadcast stride-0 over the ktile dim (the zero
   weights make the duplicated-Q ktile contribute nothing). Q,K quantized to
   fp8e4m3 (3-term hi/lo projections kept for accuracy: qk errors dominate
   the error budget).
 - P.V: lhsT ktiles = (V_hi, V_lo) fp8 split (V exact to ~0.03%; V error
   enters the output linearly on small-neff rows, so it must stay tight);
   rhs = P (exp output written directly as fp8e4m3) broadcast stride-0.
Causal masking stays post-exp via tri-mask multiplies (DVE). Output
projection stays bf16. ACT (exp, ~58us of columns + per-instr overhead),
DVE (drain copies + normalize), and PE (~75us) end up balanced at ~76us
each; the schedule manages the in-order-queue hazards: proj drain bursts
are paced with a post-copy cooldown, y-proj units live in a separate
low-priority filler queue spaced ~3us apart in exp-time (their serial
matmul->PSUM-drain chains otherwise bunch where the proj queue empties
and head-of-line-block the PE at stream turns), prologue DMAs are
ordered pair-0-first (HWDGE issues serialize at ~650ns and transfers
serialize on the DMA pipe), and junk zero-matmuls at t=0 climb the PE
p-state ramp during the DMA wait.
"""
import collections
import sys
from contextlib import ExitStack

sys.path.insert(0, "/opt/trn_rl_repo")

import ml_dtypes
import numpy as np

import concourse.bass as bass
import concourse.tile as tile
from concourse import bacc, mybir
from concourse.bass_utils import run_bass_kernel_spmd

F32 = mybir.dt.float32
BF16 = mybir.dt.bfloat16
FP8 = mybir.dt.float8e4
DR = mybir.MatmulPerfMode.DoubleRow
EXP = mybir.ActivationFunctionType.Exp
SUB = mybir.AluOpType.subtract
MUL = mybir.AluOpType.mult

WSCALE = 32.0           # host prescale on Wq/Wk/Wv for fp8 range; q,k,v come
                        # out x32, folded into the exp scale and into Wp

B, T, E, H = 2, 2048, 1024, 16
D = E // H              # 64
N_CORES = 8
GH = 4                  # heads per core
GE = GH * D             # 256 per-core projection width
SCALE = float(D) ** -0.5

TCH = 512               # projection t-chunk
NTCH = T // TCH         # 4
KC = 8                  # contraction chunks of 128 over E
KC2 = KC // 2
QB = 512                # attention q-block
NQB = T // QB           # 4
KB = 128                # attention k-block
NKB = T // KB           # 16
VSLOT = 80              # v8 per-head slot width (64 data + ones col + pad, %16)

PE_NS = 1e9 / 2.4e9     # per moving-free column (bf16)
ACT_NS = 1e9 / 1.2e9    # per free column
EXP_OVH = 217.0         # ACT per-instruction overhead (access + decode)

DEFAULT_OPTS = dict(
    s_bufs=2,
    o_bufs=2,
    pv_bufs=2,
    p_bufs=7,
    x_bufs=4,
    l_bufs=6,
    on_bufs=8,
    yst_bufs=4,
    norm_splits=1,       # normalize split count (qb < last)
    norm_splits_last=4,  # normalize split count for the last q-block
    sem_lat=400.0,       # pacing fudge: SS-end -> exp-start latency
    ret_lat=1000.0,      # pacing fudge: exp-end -> O-start latency
    lead=0.0,            # pacing margin (ns)
    end_fill=1200.0,     # filler ns pulled at each stream end (norm window)
    copy_cd=1000.0,      # ns between a proj drain copy and the next slot alloc
    y_defer=1,           # 1: Y(qb) paced into phase qb+1; 3: all saved for last phase
    qk_copy_eng="vector",
    qk_terms=3,          # x*W terms for Q/K proj (1=hh only; 3=hh,lh,hl)
    tri_eng="vector",    # engine for the post-exp causal masking multiplies
    y_tail_split=True,   # alternate last-phase y drains between DVE and ACT
    early_qp0=False,     # drain next phase's qp0 at pair-1 start (DVE queue
                         # order: its copy lands before the phase-end bursts)
    sel_drain=True,      # drain() defers y units instead of emitting them
    cd_hist=1,           # which drain-copy (1=last) gates the next slot alloc
    early_act_copies=1,  # tch < this: qk/v drain copies go to ACT (idle early)
    warmup=80,           # junk DR matmuls at t=0 to climb the PE p-state ramp
    qk1_scalar=False,    # pair-1 q/k drain copies on ACT (idle at pair turn)
    y_si=1,              # y fillers allowed in streams si < y_si
    mid_qp1=False,       # drain this phase's qp1 mid-way through stream si1
    si_interleave=False, # stream order (0,0),(1,0),(0,1),(1,1) within a phase
    q_scalar=False,      # all q drain copies on ACT (they gate phase/pair turns)
    merge_heads=False,   # interleave both heads of a pair in one stream
    y_tail_wide=True,    # tail y units alternate pv/s psum pools (s is idle)
    pipe_depth=1,        # SS->OO software pipeline lag (groups)
    y_wide=False,        # alternate y psum pools in all phases, not just tail
    y_cd=4200.0,         # min act-clk ns between y filler pulls
    carry_oo=True,       # carry pending OOs across stream boundaries
    tri_in_ss=True,      # emit diag tri-muls right after the exp (not in OO)
    x_upfront=False,     # issue all x DMAs from the prologue (with x_bufs=8)
    y_late_from=0,       # y units of phases >= this use pv_ps instead of s_ps
)


def build_program(opts=None):
    o = dict(DEFAULT_OPTS)
    if opts:
        o.update(opts)
    nc = bacc.Bacc("TRN2", target_bir_lowering=False, debug=False, num_devices=N_CORES)

    xh_d = nc.dram_tensor("xh", [E, T], FP8, kind="ExternalInput").ap()
    xl_d = nc.dram_tensor("xl", [E, T], FP8, kind="ExternalInput").ap()
    wqh_d = nc.dram_tensor("wqh", [E, GE], FP8, kind="ExternalInput").ap()
    wql_d = nc.dram_tensor("wql", [E, GE], FP8, kind="ExternalInput").ap()
    wkh_d = nc.dram_tensor("wkh", [E, GE], FP8, kind="ExternalInput").ap()
    wkl_d = nc.dram_tensor("wkl", [E, GE], FP8, kind="ExternalInput").ap()
    wvh_d = nc.dram_tensor("wvh", [E, GE], FP8, kind="ExternalInput").ap()
    wvl_d = nc.dram_tensor("wvl", [E, GE], FP8, kind="ExternalInput").ap()
    wpt_d = nc.dram_tensor("wpt", [GE, E], BF16, kind="ExternalInput").ap()
    tri_d = nc.dram_tensor("tri", [KB, KB], FP8, kind="ExternalInput").ap()
    y_d = nc.dram_tensor("y", [T, E], BF16, kind="ExternalOutput").ap()

    with tile.TileContext(nc) as tc:
        with tc.tile_pool(name="weights", bufs=1) as wpool, \
             tc.tile_pool(name="qk", bufs=1) as qkpool, \
             tc.tile_pool(name="vsb", bufs=1) as vpool, \
             tc.tile_pool(name="xin", bufs=o["x_bufs"]) as xpool, \
             tc.tile_pool(name="ptile", bufs=o["p_bufs"]) as ppool, \
             tc.tile_pool(name="lbc", bufs=o["l_bufs"]) as lpool, \
             tc.tile_pool(name="onorm", bufs=o["on_bufs"]) as onpool, \
             tc.tile_pool(name="ystage", bufs=o["yst_bufs"]) as ypool, \
             tc.tile_pool(name="s_ps", bufs=o["s_bufs"], space="PSUM") as s_ps, \
             tc.tile_pool(name="pv_ps", bufs=o["pv_bufs"], space="PSUM") as pv_ps, \
             tc.tile_pool(name="o_ps", bufs=o["o_bufs"], space="PSUM") as o_ps:
            qk_ps = v_ps = pv_ps

            nqk = 2 if o["qk_terms"] > 1 else 1
            wq_sb = [wpool.tile([128, KC2, 2, GE], FP8, name=f"wq{i}") for i in range(nqk)]
            wk_sb = [wpool.tile([128, KC2, 2, GE], FP8, name=f"wk{i}") for i in range(nqk)]
            wv_sb = [wpool.tile([128, KC2, 2, GE], FP8, name=f"wv{i}") for i in range(2)]
            wp_sb = wpool.tile([128, 2, E], BF16)
            tri_sb = wpool.tile([KB, KB], FP8)

            # Q^T per pair: [p=(h%2,d), pair, t] fp8
            qt_sb = qkpool.tile([128, 2, T], FP8)
            # K^T chunked: chunks 0 and 33 = zeros (DR ktile partner / warmup
            # operands), chunks 1+pair*16+j = K^T[:, j*128:(j+1)*128]
            kt_sb = qkpool.tile([128, 2 * NKB + 2, KB], FP8)
            # V: [p=key%128, tblock, hi/lo, head*80 + (d | ones at 64 | pad)]
            v_sb = vpool.tile([128, NKB, 2, GH * VSLOT], FP8)

            xts = [None] * NTCH  # per-tch ([hi, lo]) [128, KC2, 2, TCH] tiles

            def dr(ap3):
                # dram [rows, n] -> DoubleRow sbuf layout [p, c2, i, n]
                return ap3.rearrange("(c i p) n -> p c i n", i=2, p=128)

            def w_dma(w_sb_t, w_d):
                nc.sync.dma_start(out=w_sb_t[:], in_=dr(w_d))

            def emit_x_dma(tch):
                xts[tch] = [xpool.tile([128, KC2, 2, TCH], FP8, tag="xt",
                                       name=f"x{tch}_{hl}") for hl in range(2)]
                xsrc = [x_d[:, tch * TCH:(tch + 1) * TCH] for x_d in (xh_d, xl_d)]
                if tch == 0:
                    # prologue: HWDGE serializes DMA issue at ~650ns each and
                    # transfers serialize on the DMA pipe, so order by first
                    # use and load only the pair-0 weight columns up front:
                    # the pair-0 Q/K projection chain gates the first exp.
                    def w_half(w_sb_t, w_d, pair):
                        sl = slice(pair * 128, (pair + 1) * 128)
                        nc.sync.dma_start(out=w_sb_t[:, :, :, sl],
                                          in_=dr(w_d)[:, :, :, sl])
                    nc.sync.dma_start(out=xts[0][0][:], in_=dr(xsrc[0]))
                    w_half(wq_sb[0], wqh_d, 0)
                    if nqk > 1:
                        nc.sync.dma_start(out=xts[0][1][:], in_=dr(xsrc[1]))
                        w_half(wq_sb[1], wql_d, 0)
                    w_half(wk_sb[0], wkh_d, 0)
                    if nqk > 1:
                        w_half(wk_sb[1], wkl_d, 0)
                    w_dma(wv_sb[0], wvh_d)
                    w_dma(wv_sb[1], wvl_d)
                    if nqk == 1:
                        nc.sync.dma_start(out=xts[0][1][:], in_=dr(xsrc[1]))
                    w_half(wq_sb[0], wqh_d, 1)
                    if nqk > 1:
                        w_half(wq_sb[1], wql_d, 1)
                    w_half(wk_sb[0], wkh_d, 1)
                    if nqk > 1:
                        w_half(wk_sb[1], wkl_d, 1)
                    nc.sync.dma_start(out=tri_sb[:], in_=tri_d)
                    emit_x_dma(1)
                    nc.sync.dma_start(
                        out=wp_sb[:], in_=wpt_d.rearrange("(c p) n -> p c n", p=128))
                    if o["x_upfront"]:
                        # all x chunks issued from the prologue: needs enough
                        # x bufs that no DMA waits a tile free (the in-order
                        # SP queue would head-block the y output DMAs)
                        emit_x_dma(2)
                        emit_x_dma(3)
                else:
                    for hl in range(2):
                        nc.sync.dma_start(out=xts[tch][hl][:], in_=dr(xsrc[hl]))

            # ---- pacing state ------------------------------------------------
            clk = {"pe": 0.0, "act": 0.0, "allow_y": False, "last_y": -1e9}
            pend = collections.deque()  # carried (pt, oo, hook, after) entries

            def pop_oo():
                pt_, oo_, hook_, after_ = pend.popleft()
                if hook_ is not None:
                    hook_()
                oo_(pt_)
                if after_ is not None:
                    after_()
            copy_hist = collections.deque([-1e9] * 8, maxlen=8)
            fillers = collections.deque()    # proj units (tag, pe_ns, fn, allocs)
            fillers_y = collections.deque()  # y units: only emitted mid-stream

            def mm(pe_ns):
                clk["pe"] += pe_ns

            def emit_from(q):
                tag, pe_ns, fn, _alloc = q.popleft()
                marks = fn() or ()
                clk["pe"] += pe_ns
                if "copy" in marks:
                    copy_hist.append(clk["pe"])

            def emit_one():
                emit_from(fillers)

            def cd_blocked():
                # hold back a unit that re-allocates a shared proj psum slot
                # until the drain copy cd_hist groups back had time to land
                return clk["pe"] < copy_hist[-o["cd_hist"]] + o["copy_cd"]

            def pace(target):
                # proj fillers first; y units only mid-stream (a y matmul
                # stalls the in-order PE queue on its DVE drain copy, so they
                # must never sit ahead of a phase's first S matmuls) and
                # spaced out by act-clock so their copy chains hide under exps
                while clk["pe"] < target - o["lead"]:
                    if fillers and not (fillers[0][3] and cd_blocked()):
                        emit_from(fillers)
                    elif (fillers_y and clk["allow_y"]
                          and clk["act"] >= clk["last_y"] + o["y_cd"]):
                        clk["last_y"] = clk["act"]
                        emit_from(fillers_y)
                    else:
                        break

            def drain(tag_pred):
                while any(tag_pred(t) for t, _, _, _ in fillers):
                    emit_one()

            def qk_drain_copy(dst, src, tch=99, pair=0, is_q=False):
                if (o["qk_copy_eng"] == "scalar" or tch < o["early_act_copies"]
                        or (pair == 1 and o["qk1_scalar"])
                        or (is_q and o["q_scalar"])):
                    nc.scalar.copy(out=dst, in_=src)
                else:
                    nc.vector.tensor_copy(out=dst, in_=src)

            # ---- projection units -------------------------------------------
            TERMS = ((0, 0), (1, 0), (0, 1))  # (w hi/lo, x hi/lo): hh, lh, hl

            QK_TERMS = TERMS[:o["qk_terms"]]

            def qk_mms(ph, w_sb, tch, pair, c2):
                for ti, (wl, xl) in enumerate(QK_TERMS):
                    nc.tensor.matmul(
                        ph[:],
                        w_sb[wl][:, c2, :, pair * 128:(pair + 1) * 128],
                        xts[tch][xl][:, c2, :, :],
                        start=(c2 == 0 and ti == 0),
                        stop=(c2 == KC2 - 1 and ti == len(QK_TERMS) - 1),
                        perf_mode=DR)

            QKC = (TCH // 2) * PE_NS * o["qk_terms"]  # pe-ns per qk unit

            def q_units(tch):
                qp_h = {}

                def q_u(pair, c2):
                    def fn():
                        if c2 == 0:
                            qp_h[pair] = qk_ps.tile([128, TCH], F32, tag="pv",
                                                    name=f"q_{tch}_{pair}")
                        qk_mms(qp_h[pair], wq_sb, tch, pair, c2)
                        if c2 == KC2 - 1:
                            qk_drain_copy(
                                qt_sb[:, pair, tch * TCH:(tch + 1) * TCH],
                                qp_h[pair][:], tch, pair, is_q=True)
                            return ("copy",)
                    return fn
                return [(f"qp{pair}", QKC, q_u(pair, c2), c2 == 0)
                        for pair in range(2) for c2 in range(KC2)]

            def kv_units(tch):
                kp_h = {}

                def k_u(pair, c2):
                    def fn():
                        if c2 == 0:
                            kp_h[pair] = qk_ps.tile([128, TCH], F32, tag="pv",
                                                    name=f"k_{tch}_{pair}")
                        qk_mms(kp_h[pair], wk_sb, tch, pair, c2)
                        if c2 == KC2 - 1:
                            ch = 1 + pair * NKB + tch * (TCH // KB)
                            qk_drain_copy(
                                kt_sb[:, ch:ch + TCH // KB, :],
                                kp_h[pair].rearrange("p (c n) -> p c n", n=KB), tch, pair)
                            return ("copy",)
                    return fn

                units = [(f"kp{pair}", QKC, k_u(pair, c2), c2 == 0)
                         for pair in range(2) for c2 in range(KC2)]
                vp_h = {}

                def v_u(tsub, half):
                    def fn():
                        if half == 0:
                            vp_h[tsub] = v_ps.tile([128, GE], F32, tag="pv",
                                                   name=f"vp{tch}_{tsub}")
                        for c2 in range(2 * half, 2 * half + 2):
                            for ti, (wl, xl) in enumerate(TERMS):
                                nc.tensor.matmul(
                                    vp_h[tsub][:],
                                    xts[tch][xl][:, c2, :, tsub * KB:(tsub + 1) * KB],
                                    wv_sb[wl][:, c2, :, :],
                                    start=(c2 == 0 and ti == 0),
                                    stop=(c2 == KC2 - 1 and ti == len(TERMS) - 1),
                                    perf_mode=DR)
                        if half == 1:
                            tb = tch * (TCH // KB) + tsub
                            vsrc = vp_h[tsub].rearrange("p (h c) -> p h c", c=D)
                            vhi = v_sb.rearrange(
                                "p b i (h w) -> p b i h w", w=VSLOT)[:, tb, 0, :, 0:D]
                            vlo = v_sb.rearrange(
                                "p b i (h w) -> p b i h w", w=VSLOT)[:, tb, 1, :, 0:D]
                            if tch < o["early_act_copies"]:
                                nc.scalar.copy(out=vhi, in_=vsrc)
                            else:
                                nc.vector.tensor_copy(out=vhi, in_=vsrc)
                            nc.vector.tensor_tensor(out=vlo, in0=vsrc, in1=vhi, op=SUB)
                            return ("copy",)
                    return fn

                # shared proj psum slots: groups must stay contiguous
                k0 = [u for u in units if u[0] == "kp0"]
                k1 = [u for u in units if u[0] == "kp1"]
                vs = [("v", 3 * GE * PE_NS, v_u(t, half), half == 0)
                      for t in range(TCH // KB) for half in range(2)]
                return k0, vs, k1

            # ---- output-projection units ------------------------------------
            def y_units(qb, onorms):
                q0 = qb * QB
                units = []
                yt_h = {}
                late = qb >= o["y_late_from"]  # pv pool quiet in late phases

                def y_unit(qt, nh):
                    def fn():
                        if nh == 0:
                            yt_h[qt] = ypool.tile([128, E], BF16, tag="yt", name=f"yt{qt}")
                        if (qb == NQB - 1 and o["y_tail_wide"]) or o["y_wide"]:
                            # y units are pulled as fillers when the proj queue
                            # is empty, so pv slots are free then: alternating
                            # pools doubles the slots and compresses the
                            # copy-wait ping-pong chain at stream turns
                            pool = pv_ps if (qt + nh) % 2 else s_ps
                            yp = pool.tile([128, 512], F32,
                                           tag="pv" if (qt + nh) % 2 else "s", name="yp")
                        elif late:
                            yp = pv_ps.tile([128, 512], F32, tag="pv", name="yp")
                        else:
                            yp = s_ps.tile([128, 512], F32, tag="s", name="yp")
                        for pair in range(2):
                            nc.tensor.matmul(yp[:],
                                             onorms[pair][:, qt * 128:(qt + 1) * 128],
                                             wp_sb[:, pair, nh * 512:(nh + 1) * 512],
                                             start=(pair == 0), stop=(pair == 1))
                        ysl = yt_h[qt][:, nh * 512:(nh + 1) * 512]
                        if qb == NQB - 1 and o["y_tail_split"] and (qt + nh) % 2:
                            nc.scalar.copy(out=ysl, in_=yp[:])
                        else:
                            nc.vector.tensor_copy(out=ysl, in_=yp[:])
                        nc.sync.dma_start(
                            out=y_d[q0 + qt * 128:q0 + (qt + 1) * 128, nh * 512:(nh + 1) * 512],
                            in_=ysl)
                    return fn

                for qt in range(QB // 128):
                    for nh in range(2):
                        units.append(("y", 2 * 512 * PE_NS, y_unit(qt, nh), False))
                return units

            # ---- attention stream -------------------------------------------
            def vslot(hb, hl):
                # [128, 2, 65] hi/lo ktile view of head hb at key-block j
                def at(j):
                    base = v_sb.rearrange("p b i (h w) -> p b i h w", w=VSLOT)
                    return base[:, j, :, hb, 0:D + 1]
                return at

            def kdr(pair, j, h):
                # lhsT [64, 2, 128]: ktile 0 = zeros chunk, ktile 1 = K chunk
                c = 1 + pair * NKB + j
                return kt_sb[h * D:(h + 1) * D, 0:c + 1:c, :]

            def bcast2(ap):
                return ap.unsqueeze(1).broadcast_to([ap.shape[0], 2, ap.shape[1]])

            def bcast2p(ap):
                return ap.unsqueeze(1).broadcast_to([128, 2, ap.shape[1]])

            def normalize(o_p, onorm, h, splits=1):
                w = QB // splits
                for s in range(splits):
                    qs = slice(s * w, (s + 1) * w)
                    strip = lpool.tile([1, w], F32, tag="strip")
                    nc.vector.reciprocal(out=strip[:], in_=o_p[D:D + 1, qs])
                    lb = lpool.tile([D, w], F32, tag="lb")
                    nc.gpsimd.partition_broadcast(lb[:], strip[:])
                    nc.vector.tensor_mul(onorm[h * D:(h + 1) * D, qs], o_p[0:D, qs], lb[:])

            def stream2(qb, pair, onorm, splits, prek=None, prev_v=None):
                # both heads of the pair interleaved in one software-pipelined
                # sequence: each exp is covered by the other head's matmuls and
                # the exp chain never breaks at a head boundary
                q0 = qb * QB
                nk = (q0 + QB) // KB
                nfull = nk - 4
                ngrp = nk // 2
                o_ps_h = [o_ps.tile([D + 1, QB], F32, tag="o", name=f"o2_{h}")
                          for h in range(2)]

                def grp2(h, j2, diag):
                    bsl = slice(h * D, h * D + D)
                    hb = pair * 2 + h
                    vat = vslot(hb, 0)
                    o_p = o_ps_h[h]
                    qrhs = qt_sb[bsl, pair, q0:q0 + QB]
                    r0 = (j2 - nfull) * KB if diag else 0
                    r1 = r0 + KB
                    w1 = QB - r1

                    def ss():
                        sp = s_ps.tile([128, 2 * QB], F32, tag="s", name="sp")
                        pt = ppool.tile([128, 2 * QB], FP8, tag="p", name="pt")
                        if diag:
                            nc.tensor.matmul(sp[:, r0:QB], kdr(pair, j2, h),
                                             bcast2(qrhs[:, r0:]),
                                             start=True, stop=True, perf_mode=DR)
                            nc.tensor.matmul(sp[:, QB:QB + w1], kdr(pair, j2 + 1, h),
                                             bcast2(qrhs[:, r1:]),
                                             start=True, stop=True, perf_mode=DR)
                            mm((QB - r0 + w1) * PE_NS / 2)
                            nc.scalar.activation(out=pt[:, r0:QB + w1], in_=sp[:, r0:QB + w1],
                                                 func=EXP, scale=SCALE / (WSCALE * WSCALE))
                            clk["act"] = max(clk["act"], clk["pe"] + o["sem_lat"])                                 + (QB - r0 + w1) * ACT_NS + EXP_OVH
                        else:
                            for jj in range(2):
                                nc.tensor.matmul(sp[:, jj * QB:(jj + 1) * QB],
                                                 kdr(pair, j2 + jj, h), bcast2(qrhs),
                                                 start=True, stop=True, perf_mode=DR)
                            mm(2 * QB * PE_NS / 2)
                            nc.scalar.activation(out=pt[:], in_=sp[:], func=EXP,
                                                 scale=SCALE / (WSCALE * WSCALE))
                            clk["act"] = max(clk["act"], clk["pe"] + o["sem_lat"])                                 + 2 * QB * ACT_NS + EXP_OVH
                        return pt

                    def oo(pt):
                        if diag:
                            if not o["tri_in_ss"]:
                                tri_e = nc.gpsimd if o["tri_eng"] == "pool" else nc.vector
                                tri_e.tensor_tensor(out=pt[:, r0:r0 + KB],
                                                    in0=pt[:, r0:r0 + KB], in1=tri_sb[:], op=MUL)
                                tri_e.tensor_tensor(out=pt[:, QB:QB + KB],
                                                    in0=pt[:, QB:QB + KB], in1=tri_sb[:], op=MUL)
                            nc.tensor.matmul(o_p[:, r0:QB], vat(j2),
                                             bcast2p(pt[:, r0:QB]),
                                             start=(j2 == 0), stop=False, perf_mode=DR)
                            nc.tensor.matmul(o_p[:, r1:QB], vat(j2 + 1),
                                             bcast2p(pt[:, QB:QB + w1]),
                                             start=False, stop=(j2 + 1 == nk - 1),
                                             perf_mode=DR)
                            mm((QB - r0 + w1) * PE_NS / 2)
                        else:
                            for jj in range(2):
                                j = j2 + jj
                                nc.tensor.matmul(o_p[:], vat(j),
                                                 bcast2p(pt[:, jj * QB:(jj + 1) * QB]),
                                                 start=(j == 0), stop=False, perf_mode=DR)
                            mm(2 * QB * PE_NS / 2)
                    return ss, oo

                seq = []
                for g in range(ngrp):
                    j2 = 2 * g
                    diag = j2 >= nfull
                    for h in range(2):
                        seq.append((h, g, grp2(h, j2, diag)))
                prev = None
                for idx, (h, g, (ss, oo)) in enumerate(seq):
                    if idx == max(0, len(seq) - 4) and prek is not None:
                        prek()
                    pt = ss()
                    pace(clk["act"] + o["ret_lat"])
                    if prev is not None:
                        if prev[3] == max(0, len(seq) - 4) and prev_v is not None:
                            prev_v()
                        prev[1](prev[0])
                        if prev[4] == ngrp - 1:
                            normalize(o_ps_h[prev[2]], onorm, prev[2], splits)
                    prev = (pt, oo, h, idx, g)
                if prev[3] == max(0, len(seq) - 4) and prev_v is not None:
                    prev_v()
                prev[1](prev[0])
                normalize(o_ps_h[prev[2]], onorm, prev[2], splits)
                pace(clk["pe"] + o["end_fill"])

            def stream(qb, pair, h, onorm, splits, prek=None, prev_v=None,
                       mid=None):
                q0 = qb * QB
                nk = (q0 + QB) // KB
                nfull = nk - 4
                bsl = slice(h * D, h * D + D)
                hb = pair * 2 + h
                vat = vslot(hb, 0)
                o_p = o_ps.tile([D + 1, QB], F32, tag="o")
                qrhs = qt_sb[bsl, pair, q0:q0 + QB]

                def grp(j2, diag):
                    r0 = (j2 - nfull) * KB if diag else 0
                    r1 = r0 + KB
                    w1 = QB - r1

                    def ss():
                        sp = s_ps.tile([128, 2 * QB], F32, tag="s", name="sp")
                        pt = ppool.tile([128, 2 * QB], FP8, tag="p", name="pt")
                        if diag:
                            nc.tensor.matmul(sp[:, r0:QB], kdr(pair, j2, h),
                                             bcast2(qrhs[:, r0:]),
                                             start=True, stop=True, perf_mode=DR)
                            nc.tensor.matmul(sp[:, QB:QB + w1], kdr(pair, j2 + 1, h),
                                             bcast2(qrhs[:, r1:]),
                                             start=True, stop=True, perf_mode=DR)
                            mm((QB - r0 + w1) * PE_NS / 2)
                            nc.scalar.activation(out=pt[:, r0:QB + w1], in_=sp[:, r0:QB + w1],
                                                 func=EXP, scale=SCALE / (WSCALE * WSCALE))
                            clk["act"] = max(clk["act"], clk["pe"] + o["sem_lat"]) \
                                + (QB - r0 + w1) * ACT_NS + EXP_OVH
                            if o["tri_in_ss"]:
                                tri_e = nc.gpsimd if o["tri_eng"] == "pool" else nc.vector
                                tri_e.tensor_tensor(out=pt[:, r0:r0 + KB],
                                                    in0=pt[:, r0:r0 + KB], in1=tri_sb[:], op=MUL)
                                tri_e.tensor_tensor(out=pt[:, QB:QB + KB],
                                                    in0=pt[:, QB:QB + KB], in1=tri_sb[:], op=MUL)
                        else:
                            for jj in range(2):
                                j = j2 + jj
                                nc.tensor.matmul(sp[:, jj * QB:(jj + 1) * QB],
                                                 kdr(pair, j, h), bcast2(qrhs),
                                                 start=True, stop=True, perf_mode=DR)
                            mm(2 * QB * PE_NS / 2)
                            nc.scalar.activation(out=pt[:], in_=sp[:], func=EXP,
                                                 scale=SCALE / (WSCALE * WSCALE))
                            clk["act"] = max(clk["act"], clk["pe"] + o["sem_lat"]) \
                                + 2 * QB * ACT_NS + EXP_OVH
                        return pt

                    def oo(pt):
                        if diag:
                            if not o["tri_in_ss"]:
                                tri_e = nc.gpsimd if o["tri_eng"] == "pool" else nc.vector
                                tri_e.tensor_tensor(out=pt[:, r0:r0 + KB],
                                                    in0=pt[:, r0:r0 + KB], in1=tri_sb[:], op=MUL)
                                tri_e.tensor_tensor(out=pt[:, QB:QB + KB],
                                                    in0=pt[:, QB:QB + KB], in1=tri_sb[:], op=MUL)
                            nc.tensor.matmul(o_p[:, r0:QB], vat(j2),
                                             bcast2p(pt[:, r0:QB]),
                                             start=(j2 == 0), stop=False, perf_mode=DR)
                            nc.tensor.matmul(o_p[:, r1:QB], vat(j2 + 1),
                                             bcast2p(pt[:, QB:QB + w1]),
                                             start=False, stop=(j2 + 1 == nk - 1),
                                             perf_mode=DR)
                            mm((QB - r0 + w1) * PE_NS / 2)
                        else:
                            for jj in range(2):
                                j = j2 + jj
                                nc.tensor.matmul(o_p[:], vat(j),
                                                 bcast2p(pt[:, jj * QB:(jj + 1) * QB]),
                                                 start=(j == 0), stop=False, perf_mode=DR)
                            mm(2 * QB * PE_NS / 2)
                    return ss, oo

                groups = [grp(j2, False) for j2 in range(0, nfull, 2)] \
                    + [grp(j2, True) for j2 in range(nfull, nk, 2)]
                # software-pipelined: SS/exp of groups g+1..g+depth run before
                # OO of g so the next S matmuls (plus fillers) cover the exp
                # latency; the queue is GLOBAL, so a stream's last OO(s) and
                # its normalize are emitted after the NEXT stream's first S
                # matmuls -- the ACT queue never drains at a stream turn
                ndiag = len(groups) - 2  # first group whose SS reads this tch's K
                depth = o["pipe_depth"]
                for gi, (ss, oo) in enumerate(groups):
                    if gi == min(1, ndiag) and mid is not None:
                        mid()
                    if gi == ndiag and prek is not None:
                        prek()
                    pt = ss()
                    pace(clk["act"] + o["ret_lat"])
                    if len(pend) >= depth:
                        pop_oo()
                    hook = prev_v if gi == ndiag else None
                    after = None
                    if gi == len(groups) - 1:
                        def after(o_p=o_p, onorm=onorm, h=h, splits=splits):
                            normalize(o_p, onorm, h, splits)
                    pend.append((pt, oo, hook, after))
                if not o["carry_oo"]:
                    while pend:
                        pop_oo()
                pace(clk["pe"] + o["end_fill"])

            # ---- main schedule ----------------------------------------------
            nc.vector.memset(kt_sb[:, 0, :], 0.0)
            nc.vector.memset(kt_sb[:, 2 * NKB + 1, :], 0.0)
            if o["warmup"]:
                # climb the PE p-state ramp on junk zero matmuls while the
                # prologue DMAs land; results are discarded
                wu_ps = s_ps.tile([128, 2 * QB], F32, tag="s", name="wu")
                wu_lhs = kt_sb[0:64, 0::(2 * NKB + 1), :]
                wu_rhs = kt_sb[0:64, 0, :].unsqueeze(1).broadcast_to([64, 2, KB])
                for _ in range(o["warmup"]):
                    nc.tensor.matmul(wu_ps[:, 0:KB], wu_lhs, wu_rhs,
                                     start=True, stop=True, perf_mode=DR)
            emit_x_dma(0)
            vv = v_sb.rearrange("p b i (h w) -> p b i h w", w=VSLOT)
            nc.vector.memset(vv[:, :, 0, :, D:D + 1], 1.0)
            nc.vector.memset(vv[:, :, 1, :, D:D + 1], 0.0)

            def queue_proj(tch):
                qs = q_units(tch)
                q0 = [u for u in qs if u[0] == "qp0"]
                q1 = [u for u in qs if u[0] == "qp1"]
                k0, vs, k1 = kv_units(tch)
                for t, c, f, a in q0 + k0 + vs + q1 + k1:
                    fillers.append((f"{t}@{tch}", c, f, a))

            pending_y = []
            for qb in range(NQB):
                if qb == 0:
                    queue_proj(0)
                if 0 < qb < NQB - 1 and not o["x_upfront"]:
                    emit_x_dma(qb + 1)
                # this phase's Q (and for streams' diagonals, K/V) must be
                # emitted before the attention that reads it
                drain(lambda t: t == f"qp0@{qb}")
                if qb < NQB - 1:
                    queue_proj(qb + 1)
                onorms = [onpool.tile([128, QB], BF16, tag="on", name=f"on{qb}_{i}") for i in range(2)]
                splits = o["norm_splits_last"] if qb == NQB - 1 else o["norm_splits"]
                if o["merge_heads"]:
                    for si, pair in enumerate((0, 1)):
                        clk["allow_y"] = si < 1 or qb == NQB - 1
                        if pair == 1:
                            drain(lambda t: t == f"qp1@{qb}")
                            if o["early_qp0"] and qb < NQB - 1:
                                drain(lambda t: t == f"qp0@{qb + 1}")
                        prek = lambda p=pair: drain(lambda t: t == f"kp{p}@{qb}")
                        prev_v = (lambda: drain(
                            lambda t: t == f"v@{qb}")) if pair == 0 else None
                        stream2(qb, pair, onorms[pair], splits, prek=prek,
                                prev_v=prev_v)
                        if si == 0 and pending_y and (
                                qb == NQB - 1 or o["y_defer"] == 1):
                            fillers_y.extend(pending_y)
                            pending_y = []
                    pending_y = pending_y + y_units(qb, onorms)
                    continue
                si_order = ((0, 0), (1, 0), (0, 1), (1, 1)) if o["si_interleave"] \
                    else ((0, 0), (0, 1), (1, 0), (1, 1))
                for si, (pair, h) in enumerate(si_order):
                    clk["allow_y"] = si < o["y_si"] or qb == NQB - 1
                    if pair == 1 and h == 0:
                        drain(lambda t: t == f"qp1@{qb}")
                        if o["early_qp0"] and qb < NQB - 1 and not o["si_interleave"]:
                            drain(lambda t: t == f"qp0@{qb + 1}")
                    if o["si_interleave"] and si == 2 and o["early_qp0"] and qb < NQB - 1:
                        drain(lambda t: t == f"qp0@{qb + 1}")
                    prek = (lambda p=pair: drain(
                        lambda t: t == f"kp{p}@{qb}")) if h == 0 else None
                    prev_v = (lambda: drain(
                        lambda t: t == f"v@{qb}")) if (pair == 0 and h == 0) else None
                    def mid_fn(q=qb):
                        drain(lambda t: t == f"qp1@{q}")
                        if o["early_qp0"] and q < NQB - 1:
                            drain(lambda t: t == f"qp0@{q + 1}")
                    mid = mid_fn if (o["mid_qp1"] and si == 1) else None
                    stream(qb, pair, h, onorms[pair], splits, prek=prek, prev_v=prev_v,
                           mid=mid)
                    if si == 0 and pending_y and (
                            qb == NQB - 1 or o["y_defer"] == 1):
                        fillers_y.extend(pending_y)
                        pending_y = []
                pending_y = pending_y + y_units(qb, onorms)
            while pend:
                pop_oo()
            clk["allow_y"] = True
            while fillers or fillers_y:
                emit_from(fillers if fillers else fillers_y)
            for _, _, fn, _a in pending_y:
                fn()

    nc.compile()
    return nc


_NC = {}


def _get_program(opts=None):
    key = tuple(sorted((opts or {}).items()))
    if key not in _NC:
        _NC[key] = build_program(opts)
    return _NC[key]


def _make_in_maps(x, Wq, Wk, Wv, Wp):
    bf = ml_dtypes.bfloat16
    f8 = ml_dtypes.float8_e4m3
    x32 = np.asarray(x, np.float32)
    xh = x32.astype(f8)
    xl = (x32 - xh.astype(np.float32)).astype(f8)

    def wsplit(W):
        wt = np.asarray(W, np.float32).T * WSCALE
        hi = wt.astype(f8)
        lo = (wt - hi.astype(np.float32)).astype(f8)
        return hi, lo

    wqh, wql = wsplit(Wq)
    wkh, wkl = wsplit(Wk)
    wvh, wvl = wsplit(Wv)
    wpt = np.ascontiguousarray((np.asarray(Wp, np.float32).T / WSCALE).astype(bf))
    tri = (np.arange(KB)[:, None] <= np.arange(KB)[None, :]).astype(f8)
    in_maps = []
    for c in range(N_CORES):
        b, hg = c // 4, c % 4
        gsl = slice(hg * GE, (hg + 1) * GE)
        in_maps.append({
            "xh": np.ascontiguousarray(xh[b].T),
            "xl": np.ascontiguousarray(xl[b].T),
            "wqh": np.ascontiguousarray(wqh[:, gsl]),
            "wql": np.ascontiguousarray(wql[:, gsl]),
            "wkh": np.ascontiguousarray(wkh[:, gsl]),
            "wkl": np.ascontiguousarray(wkl[:, gsl]),
            "wvh": np.ascontiguousarray(wvh[:, gsl]),
            "wvl": np.ascontiguousarray(wvl[:, gsl]),
            "wpt": np.ascontiguousarray(wpt[gsl, :]),
            "tri": tri,
        })
    return in_maps


def run_cores(x, Wq, Wk, Wv, Wp, bp, **spmd_kwargs):
    """Run the 8-core program; returns (y_full, BassKernelResults)."""
    nc = _get_program()
    in_maps = _make_in_maps(x, Wq, Wk, Wv, Wp)
    res = run_bass_kernel_spmd(nc, in_maps, list(range(N_CORES)), **spmd_kwargs)
    parts = [res.results[c]["y"] for c in range(N_CORES)]
    y = np.empty((B, T, E), np.float32)
    for b in range(B):
        acc = parts[4 * b].astype(np.float32)
        for hg in range(1, 4):
            acc = acc + parts[4 * b + hg].astype(np.float32)
        y[b] = acc
    y += np.asarray(bp, np.float32)[None, None, :]
    return y, res


def kernel(x, Wq, Wk, Wv, Wp, bp):
    y, _ = run_cores(x, Wq, Wk, Wv, Wp, bp)
    return y


def bench(x, Wq, Wk, Wv, Wp, bp, iters=12):
    """Time repeated on-device executions of the compiled program."""
    import time

    import jax
    import numpy as np_
    from jax.experimental.shard_map import shard_map
    from jax.sharding import Mesh, NamedSharding, PartitionSpec

    from concourse import bass2jax, mybir as mb

    nc = _get_program()
    in_maps = _make_in_maps(x, Wq, Wk, Wv, Wp)
    n_cores = N_CORES
    bass2jax.install_neuronx_cc_hook()

    partition_name = nc.partition_id_tensor.name if nc.partition_id_tensor else None
    in_names, out_names, out_avals, zero_outs = [], [], [], []
    for alloc in nc.m.functions[0].allocations:
        if not isinstance(alloc, mb.MemoryLocationSet):
            continue
        name = alloc.memorylocations[0].name
        if alloc.kind == "ExternalInput":
            if name != partition_name:
                in_names.append(name)
        elif alloc.kind == "ExternalOutput":
            out_names.append(name)
            shape = tuple(alloc.tensor_shape)
            dtype = mb.dt.np(alloc.dtype)
            out_avals.append(jax.core.ShapedArray(shape, dtype))
            zero_outs.append(np_.zeros(shape, dtype))
    n_params = len(in_names)
    all_in_names = in_names + out_names
    if partition_name is not None:
        all_in_names = all_in_names + [partition_name]

    def _body(*args):
        operands = list(args)
        if partition_name is not None:
            operands.append(bass2jax.partition_id_tensor())
        outs = bass2jax._bass_exec_p.bind(
            *operands,
            out_avals=tuple(out_avals),
            in_names=tuple(all_in_names),
            out_names=tuple(out_names),
            lowering_input_output_aliases=(),
            sim_require_finite=True,
            sim_require_nnan=True,
            nc=nc,
        )
        return tuple(outs)

    devices = jax.devices()[:n_cores]
    mesh = Mesh(np_.asarray(devices), ("core",))
    donate = tuple(range(n_params, n_params + len(out_names)))
    sharded = jax.jit(
        shard_map(_body, mesh=mesh,
                  in_specs=(PartitionSpec("core"),) * (n_params + len(out_names)),
                  out_specs=(PartitionSpec("core"),) * len(out_names),
                  check_rep=False),
        donate_argnums=donate, keep_unused=True,
    )
    sh = NamedSharding(mesh, PartitionSpec("core"))
    concat_in = [
        jax.device_put(
            np_.concatenate([np_.asarray(in_maps[c][nm]) for c in range(n_cores)], axis=0), sh)
        for nm in in_names
    ]
    zeros_np = [np_.zeros((n_cores * z.shape[0], *z.shape[1:]), z.dtype) for z in zero_outs]

    times = []
    out_arrs = None
    for it in range(iters):
        dz = [jax.device_put(z, sh) for z in zeros_np]
        jax.block_until_ready(dz)
        t0 = time.perf_counter()
        out_arrs = sharded(*concat_in, *dz)
        jax.block_until_ready(out_arrs)
        times.append(time.perf_counter() - t0)

    parts = [
        np_.asarray(out_arrs[i]).reshape(n_cores, *out_avals[i].shape)
        for i, nm in enumerate(out_names)
    ]
    yi = out_names.index("y")
    y = np_.empty((B, T, E), np_.float32)
    for b in range(B):
        acc = parts[yi][4 * b].astype(np_.float32)
        for hg in range(1, 4):
            acc = acc + parts[yi][4 * b + hg].astype(np_.float32)
        y[b] = acc
    y += np_.asarray(bp, np_.float32)[None, None, :]
    return y, times


# revision 69
# speedup vs baseline: 1.0853x; 1.0001x over previous
"""Multi-head causal attention (B=2, T=2048, E=1024, H=16, D=64) on 8 trn2 cores.

Sharding: core c -> batch b = c // 4, head-group hg = c % 4 (4 heads each).
Per-core: QKV projections for its 4 heads, causal flash attention in
transposed-score layout (S^T[k,q]; softmax denominator folded into a
ones-augmented V matmul), row-parallel output projection producing a partial
[T, E] output. Host sums the 4 partials per batch and adds the bias.

v3: all attention matmuls in fp8 DoubleRow mode (0.5 cyc/col in the cost
model, vs 1.0 for bf16), with the DR 2-ktile layout satisfied without any
partition repacking:
 - S = K^T.Q: lhsT ktiles = (zeros-chunk, K-chunk) inside one kt tile via a
   step-sliced AP; rhs = Q broadcast stride-0 over the ktile dim (the zero
   weights make the duplicated-Q ktile contribute nothing). Q,K quantized to
   fp8e4m3 (3-term hi/lo projections kept for accuracy: qk errors dominate
   the error budget).
 - P.V: lhsT ktiles = (V_hi, V_lo) fp8 split (V exact to ~0.03%; V error
   enters the output linearly on small-neff rows, so it must stay tight);
   rhs = P (exp output written directly as fp8e4m3) broadcast stride-0.
Causal masking stays post-exp via tri-mask multiplies (DVE). Output
projection stays bf16. ACT (exp, ~58us of columns + per-instr overhead),
DVE (drain copies + normalize), and PE (~75us) end up balanced at ~76us
each; the schedule manages the in-order-queue hazards: proj drain bursts
are paced with a post-copy cooldown, y-proj units live in a separate
low-priority filler queue spaced ~3us apart in exp-time (their serial
matmul->PSUM-drain chains otherwise bunch where the proj queue empties
and head-of-line-block the PE at stream turns), prologue DMAs are
ordered pair-0-first (HWDGE issues serialize at ~650ns and transfers
serialize on the DMA pipe), and junk zero-matmuls at t=0 climb the PE
p-state ramp during the DMA wait.
"""
import collections
import sys
from contextlib import ExitStack

sys.path.insert(0, "/opt/trn_rl_repo")

import ml_dtypes
import numpy as np

import concourse.bass as bass
import concourse.tile as tile
from concourse import bacc, mybir
from concourse.bass_utils import run_bass_kernel_spmd

F32 = mybir.dt.float32
BF16 = mybir.dt.bfloat16
FP8 = mybir.dt.float8e4
DR = mybir.MatmulPerfMode.DoubleRow
EXP = mybir.ActivationFunctionType.Exp
SUB = mybir.AluOpType.subtract
MUL = mybir.AluOpType.mult

WSCALE = 32.0           # host prescale on Wq/Wk/Wv for fp8 range; q,k,v come
                        # out x32, folded into the exp scale and into Wp

B, T, E, H = 2, 2048, 1024, 16
D = E // H              # 64
N_CORES = 8
GH = 4                  # heads per core
GE = GH * D             # 256 per-core projection width
SCALE = float(D) ** -0.5

TCH = 512               # projection t-chunk
NTCH = T // TCH         # 4
KC = 8                  # contraction chunks of 128 over E
KC2 = KC // 2
QB = 512                # attention q-block
NQB = T // QB           # 4
KB = 128                # attention k-block
NKB = T // KB           # 16
VSLOT = 80              # v8 per-head slot width (64 data + ones col + pad, %16)

PE_NS = 1e9 / 2.4e9     # per moving-free column (bf16)
ACT_NS = 1e9 / 1.2e9    # per free column
EXP_OVH = 217.0         # ACT per-instruction overhead (access + decode)

DEFAULT_OPTS = dict(
    s_bufs=2,
    o_bufs=2,
    pv_bufs=2,
    p_bufs=7,
    x_bufs=4,
    l_bufs=6,
    on_bufs=8,
    yst_bufs=4,
    norm_splits=1,       # normalize split count (qb < last)
    norm_splits_last=4,  # normalize split count for the last q-block
    sem_lat=400.0,       # pacing fudge: SS-end -> exp-start latency
    ret_lat=1000.0,      # pacing fudge: exp-end -> O-start latency
    lead=0.0,            # pacing margin (ns)
    end_fill=1200.0,     # filler ns pulled at each stream end (norm window)
    copy_cd=900.0,       # ns between a proj drain copy and the next slot alloc
    y_defer=1,           # 1: Y(qb) paced into phase qb+1; 3: all saved for last phase
    qk_copy_eng="vector",
    qk_terms=3,          # x*W terms for Q/K proj (1=hh only; 3=hh,lh,hl)
    tri_eng="vector",    # engine for the post-exp causal masking multiplies
    y_tail_split=True,   # alternate last-phase y drains between DVE and ACT
    early_qp0=False,     # drain next phase's qp0 at pair-1 start (DVE queue
                         # order: its copy lands before the phase-end bursts)
    sel_drain=True,      # drain() defers y units instead of emitting them
    cd_hist=1,           # which drain-copy (1=last) gates the next slot alloc
    early_act_copies=1,  # tch < this: qk/v drain copies go to ACT (idle early)
    warmup=80,           # junk DR matmuls at t=0 to climb the PE p-state ramp
    qk1_scalar=False,    # pair-1 q/k drain copies on ACT (idle at pair turn)
    y_si=1,              # y fillers allowed in streams si < y_si
    mid_qp1=False,       # drain this phase's qp1 mid-way through stream si1
    si_interleave=False, # stream order (0,0),(1,0),(0,1),(1,1) within a phase
    q_scalar=False,      # all q drain copies on ACT (they gate phase/pair turns)
    merge_heads=False,   # interleave both heads of a pair in one stream
    y_tail_wide=True,    # tail y units alternate pv/s psum pools (s is idle)
    pipe_depth=1,        # SS->OO software pipeline lag (groups)
    y_wide=False,        # alternate y psum pools in all phases, not just tail
    y_cd=4400.0,         # min act-clk ns between y filler pulls
    carry_oo=True,       # carry pending OOs across stream boundaries
    tri_in_ss=True,      # emit diag tri-muls right after the exp (not in OO)
    x_upfront=False,     # issue all x DMAs from the prologue (with x_bufs=8)
    y_late_from=0,       # y units of phases >= this use pv_ps instead of s_ps
)


def build_program(opts=None):
    o = dict(DEFAULT_OPTS)
    if opts:
        o.update(opts)
    nc = bacc.Bacc("TRN2", target_bir_lowering=False, debug=False, num_devices=N_CORES)

    xh_d = nc.dram_tensor("xh", [E, T], FP8, kind="ExternalInput").ap()
    xl_d = nc.dram_tensor("xl", [E, T], FP8, kind="ExternalInput").ap()
    wqh_d = nc.dram_tensor("wqh", [E, GE], FP8, kind="ExternalInput").ap()
    wql_d = nc.dram_tensor("wql", [E, GE], FP8, kind="ExternalInput").ap()
    wkh_d = nc.dram_tensor("wkh", [E, GE], FP8, kind="ExternalInput").ap()
    wkl_d = nc.dram_tensor("wkl", [E, GE], FP8, kind="ExternalInput").ap()
    wvh_d = nc.dram_tensor("wvh", [E, GE], FP8, kind="ExternalInput").ap()
    wvl_d = nc.dram_tensor("wvl", [E, GE], FP8, kind="ExternalInput").ap()
    wpt_d = nc.dram_tensor("wpt", [GE, E], BF16, kind="ExternalInput").ap()
    tri_d = nc.dram_tensor("tri", [KB, KB], FP8, kind="ExternalInput").ap()
    y_d = nc.dram_tensor("y", [T, E], BF16, kind="ExternalOutput").ap()

    with tile.TileContext(nc) as tc:
        with tc.tile_pool(name="weights", bufs=1) as wpool, \
             tc.tile_pool(name="qk", bufs=1) as qkpool, \
             tc.tile_pool(name="vsb", bufs=1) as vpool, \
             tc.tile_pool(name="xin", bufs=o["x_bufs"]) as xpool, \
             tc.tile_pool(name="ptile", bufs=o["p_bufs"]) as ppool, \
             tc.tile_pool(name="lbc", bufs=o["l_bufs"]) as lpool, \
             tc.tile_pool(name="onorm", bufs=o["on_bufs"]) as onpool, \
             tc.tile_pool(name="ystage", bufs=o["yst_bufs"]) as ypool, \
             tc.tile_pool(name="s_ps", bufs=o["s_bufs"], space="PSUM") as s_ps, \
             tc.tile_pool(name="pv_ps", bufs=o["pv_bufs"], space="PSUM") as pv_ps, \
             tc.tile_pool(name="o_ps", bufs=o["o_bufs"], space="PSUM") as o_ps:
            qk_ps = v_ps = pv_ps

            nqk = 2 if o["qk_terms"] > 1 else 1
            wq_sb = [wpool.tile([128, KC2, 2, GE], FP8, name=f"wq{i}") for i in range(nqk)]
            wk_sb = [wpool.tile([128, KC2, 2, GE], FP8, name=f"wk{i}") for i in range(nqk)]
            wv_sb = [wpool.tile([128, KC2, 2, GE], FP8, name=f"wv{i}") for i in range(2)]
            wp_sb = wpool.tile([128, 2, E], BF16)
            tri_sb = wpool.tile([KB, KB], FP8)

            # Q^T per pair: [p=(h%2,d), pair, t] fp8
            qt_sb = qkpool.tile([128, 2, T], FP8)
            # K^T chunked: chunks 0 and 33 = zeros (DR ktile partner / warmup
            # operands), chunks 1+pair*16+j = K^T[:, j*128:(j+1)*128]
            kt_sb = qkpool.tile([128, 2 * NKB + 2, KB], FP8)
            # V: [p=key%128, tblock, hi/lo, head*80 + (d | ones at 64 | pad)]
            v_sb = vpool.tile([128, NKB, 2, GH * VSLOT], FP8)

            xts = [None] * NTCH  # per-tch ([hi, lo]) [128, KC2, 2, TCH] tiles

            def dr(ap3):
                # dram [rows, n] -> DoubleRow sbuf layout [p, c2, i, n]
                return ap3.rearrange("(c i p) n -> p c i n", i=2, p=128)

            def w_dma(w_sb_t, w_d):
                nc.sync.dma_start(out=w_sb_t[:], in_=dr(w_d))

            def emit_x_dma(tch):
                xts[tch] = [xpool.tile([128, KC2, 2, TCH], FP8, tag="xt",
                                       name=f"x{tch}_{hl}") for hl in range(2)]
                xsrc = [x_d[:, tch * TCH:(tch + 1) * TCH] for x_d in (xh_d, xl_d)]
                if tch == 0:
                    # prologue: HWDGE serializes DMA issue at ~650ns each and
                    # transfers serialize on the DMA pipe, so order by first
                    # use and load only the pair-0 weight columns up front:
                    # the pair-0 Q/K projection chain gates the first exp.
                    def w_half(w_sb_t, w_d, pair):
                        sl = slice(pair * 128, (pair + 1) * 128)
                        nc.sync.dma_start(out=w_sb_t[:, :, :, sl],
                                          in_=dr(w_d)[:, :, :, sl])
                    nc.sync.dma_start(out=xts[0][0][:], in_=dr(xsrc[0]))
                    w_half(wq_sb[0], wqh_d, 0)
                    if nqk > 1:
                        nc.sync.dma_start(out=xts[0][1][:], in_=dr(xsrc[1]))
                        w_half(wq_sb[1], wql_d, 0)
                    w_half(wk_sb[0], wkh_d, 0)
                    if nqk > 1:
                        w_half(wk_sb[1], wkl_d, 0)
                    w_dma(wv_sb[0], wvh_d)
                    w_dma(wv_sb[1], wvl_d)
                    if nqk == 1:
                        nc.sync.dma_start(out=xts[0][1][:], in_=dr(xsrc[1]))
                    w_half(wq_sb[0], wqh_d, 1)
                    if nqk > 1:
                        w_half(wq_sb[1], wql_d, 1)
                    w_half(wk_sb[0], wkh_d, 1)
                    if nqk > 1:
                        w_half(wk_sb[1], wkl_d, 1)
                    nc.sync.dma_start(out=tri_sb[:], in_=tri_d)
                    emit_x_dma(1)
                    nc.sync.dma_start(
                        out=wp_sb[:], in_=wpt_d.rearrange("(c p) n -> p c n", p=128))
                    if o["x_upfront"]:
                        # all x chunks issued from the prologue: needs enough
                        # x bufs that no DMA waits a tile free (the in-order
                        # SP queue would head-block the y output DMAs)
                        emit_x_dma(2)
                        emit_x_dma(3)
                else:
                    for hl in range(2):
                        nc.sync.dma_start(out=xts[tch][hl][:], in_=dr(xsrc[hl]))

            # ---- pacing state ------------------------------------------------
            clk = {"pe": 0.0, "act": 0.0, "allow_y": False, "last_y": -1e9}
            pend = collections.deque()  # carried (pt, oo, hook, after) entries

            def pop_oo():
                pt_, oo_, hook_, after_ = pend.popleft()
                if hook_ is not None:
                    hook_()
                oo_(pt_)
                if after_ is not None:
                    after_()
            copy_hist = collections.deque([-1e9] * 8, maxlen=8)
            fillers = collections.deque()    # proj units (tag, pe_ns, fn, allocs)
            fillers_y = collections.deque()  # y units: only emitted mid-stream

            def mm(pe_ns):
                clk["pe"] += pe_ns

            def emit_from(q):
                tag, pe_ns, fn, _alloc = q.popleft()
                marks = fn() or ()
                clk["pe"] += pe_ns
                if "copy" in marks:
                    copy_hist.append(clk["pe"])

            def emit_one():
                emit_from(fillers)

            def cd_blocked():
                # hold back a unit that re-allocates a shared proj psum slot
                # until the drain copy cd_hist groups back had time to land
                return clk["pe"] < copy_hist[-o["cd_hist"]] + o["copy_cd"]

            def pace(target):
                # proj fillers first; y units only mid-stream (a y matmul
                # stalls the in-order PE queue on its DVE drain copy, so they
                # must never sit ahead of a phase's first S matmuls) and
                # spaced out by act-clock so their copy chains hide under exps
                while clk["pe"] < target - o["lead"]:
                    if fillers and not (fillers[0][3] and cd_blocked()):
                        emit_from(fillers)
                    elif (fillers_y and clk["allow_y"]
                          and clk["act"] >= clk["last_y"] + o["y_cd"]):
                        clk["last_y"] = clk["act"]
                        emit_from(fillers_y)
                    else:
                        break

            def drain(tag_pred):
                while any(tag_pred(t) for t, _, _, _ in fillers):
                    emit_one()

            def qk_drain_copy(dst, src, tch=99, pair=0, is_q=False):
                if (o["qk_copy_eng"] == "scalar" or tch < o["early_act_copies"]
                        or (pair == 1 and o["qk1_scalar"])
                        or (is_q and o["q_scalar"])):
                    nc.scalar.copy(out=dst, in_=src)
                else:
                    nc.vector.tensor_copy(out=dst, in_=src)

            # ---- projection units -------------------------------------------
            TERMS = ((0, 0), (1, 0), (0, 1))  # (w hi/lo, x hi/lo): hh, lh, hl

            QK_TERMS = TERMS[:o["qk_terms"]]

            def qk_mms(ph, w_sb, tch, pair, c2):
                for ti, (wl, xl) in enumerate(QK_TERMS):
                    nc.tensor.matmul(
                        ph[:],
                        w_sb[wl][:, c2, :, pair * 128:(pair + 1) * 128],
                        xts[tch][xl][:, c2, :, :],
                        start=(c2 == 0 and ti == 0),
                        stop=(c2 == KC2 - 1 and ti == len(QK_TERMS) - 1),
                        perf_mode=DR)

            QKC = (TCH // 2) * PE_NS * o["qk_terms"]  # pe-ns per qk unit

            def q_units(tch):
                qp_h = {}

                def q_u(pair, c2):
                    def fn():
                        if c2 == 0:
                            qp_h[pair] = qk_ps.tile([128, TCH], F32, tag="pv",
                                                    name=f"q_{tch}_{pair}")
                        qk_mms(qp_h[pair], wq_sb, tch, pair, c2)
                        if c2 == KC2 - 1:
                            qk_drain_copy(
                                qt_sb[:, pair, tch * TCH:(tch + 1) * TCH],
                                qp_h[pair][:], tch, pair, is_q=True)
                            return ("copy",)
                    return fn
                return [(f"qp{pair}", QKC, q_u(pair, c2), c2 == 0)
                        for pair in range(2) for c2 in range(KC2)]

            def kv_units(tch):
                kp_h = {}

                def k_u(pair, c2):
                    def fn():
                        if c2 == 0:
                            kp_h[pair] = qk_ps.tile([128, TCH], F32, tag="pv",
                                                    name=f"k_{tch}_{pair}")
                        qk_mms(kp_h[pair], wk_sb, tch, pair, c2)
                        if c2 == KC2 - 1:
                            ch = 1 + pair * NKB + tch * (TCH // KB)
                            qk_drain_copy(
                                kt_sb[:, ch:ch + TCH // KB, :],
                                kp_h[pair].rearrange("p (c n) -> p c n", n=KB), tch, pair)
                            return ("copy",)
                    return fn

                units = [(f"kp{pair}", QKC, k_u(pair, c2), c2 == 0)
                         for pair in range(2) for c2 in range(KC2)]
                vp_h = {}

                def v_u(tsub, half):
                    def fn():
                        if half == 0:
                            vp_h[tsub] = v_ps.tile([128, GE], F32, tag="pv",
                                                   name=f"vp{tch}_{tsub}")
                        for c2 in range(2 * half, 2 * half + 2):
                            for ti, (wl, xl) in enumerate(TERMS):
                                nc.tensor.matmul(
                                    vp_h[tsub][:],
                                    xts[tch][xl][:, c2, :, tsub * KB:(tsub + 1) * KB],
                                    wv_sb[wl][:, c2, :, :],
                                    start=(c2 == 0 and ti == 0),
                                    stop=(c2 == KC2 - 1 and ti == len(TERMS) - 1),
                                    perf_mode=DR)
                        if half == 1:
                            tb = tch * (TCH // KB) + tsub
                            vsrc = vp_h[tsub].rearrange("p (h c) -> p h c", c=D)
                            vhi = v_sb.rearrange(
                                "p b i (h w) -> p b i h w", w=VSLOT)[:, tb, 0, :, 0:D]
                            vlo = v_sb.rearrange(
                                "p b i (h w) -> p b i h w", w=VSLOT)[:, tb, 1, :, 0:D]
                            if tch < o["early_act_copies"]:
                                nc.scalar.copy(out=vhi, in_=vsrc)
                            else:
                                nc.vector.tensor_copy(out=vhi, in_=vsrc)
                            nc.vector.tensor_tensor(out=vlo, in0=vsrc, in1=vhi, op=SUB)
                            return ("copy",)
                    return fn

                # shared proj psum slots: groups must stay contiguous
                k0 = [u for u in units if u[0] == "kp0"]
                k1 = [u for u in units if u[0] == "kp1"]
                vs = [("v", 3 * GE * PE_NS, v_u(t, half), half == 0)
                      for t in range(TCH // KB) for half in range(2)]
                return k0, vs, k1

            # ---- output-projection units ------------------------------------
            def y_units(qb, onorms):
                q0 = qb * QB
                units = []
                yt_h = {}
                late = qb >= o["y_late_from"]  # pv pool quiet in late phases

                def y_unit(qt, nh):
                    def fn():
                        if nh == 0:
                            yt_h[qt] = ypool.tile([128, E], BF16, tag="yt", name=f"yt{qt}")
                        if (qb == NQB - 1 and o["y_tail_wide"]) or o["y_wide"]:
                            # y units are pulled as fillers when the proj queue
                            # is empty, so pv slots are free then: alternating
                            # pools doubles the slots and compresses the
                            # copy-wait ping-pong chain at stream turns
                            pool = pv_ps if (qt + nh) % 2 else s_ps
                            yp = pool.tile([128, 512], F32,
                                           tag="pv" if (qt + nh) % 2 else "s", name="yp")
                        elif late:
                            yp = pv_ps.tile([128, 512], F32, tag="pv", name="yp")
                        else:
                            yp = s_ps.tile([128, 512], F32, tag="s", name="yp")
                        for pair in range(2):
                            nc.tensor.matmul(yp[:],
                                             onorms[pair][:, qt * 128:(qt + 1) * 128],
                                             wp_sb[:, pair, nh * 512:(nh + 1) * 512],
                                             start=(pair == 0), stop=(pair == 1))
                        ysl = yt_h[qt][:, nh * 512:(nh + 1) * 512]
                        if qb == NQB - 1 and o["y_tail_split"] and (qt + nh) % 2:
                            nc.scalar.copy(out=ysl, in_=yp[:])
                        else:
                            nc.vector.tensor_copy(out=ysl, in_=yp[:])
                        nc.sync.dma_start(
                            out=y_d[q0 + qt * 128:q0 + (qt + 1) * 128, nh * 512:(nh + 1) * 512],
                            in_=ysl)
                    return fn

                for qt in range(QB // 128):
                    for nh in range(2):
                        units.append(("y", 2 * 512 * PE_NS, y_unit(qt, nh), False))
                return units

            # ---- attention stream -------------------------------------------
            def vslot(hb, hl):
                # [128, 2, 65] hi/lo ktile view of head hb at key-block j
                def at(j):
                    base = v_sb.rearrange("p b i (h w) -> p b i h w", w=VSLOT)
                    return base[:, j, :, hb, 0:D + 1]
                return at

            def kdr(pair, j, h):
                # lhsT [64, 2, 128]: ktile 0 = zeros chunk, ktile 1 = K chunk
                c = 1 + pair * NKB + j
                return kt_sb[h * D:(h + 1) * D, 0:c + 1:c, :]

            def bcast2(ap):
                return ap.unsqueeze(1).broadcast_to([ap.shape[0], 2, ap.shape[1]])

            def bcast2p(ap):
                return ap.unsqueeze(1).broadcast_to([128, 2, ap.shape[1]])

            def normalize(o_p, onorm, h, splits=1):
                w = QB // splits
                for s in range(splits):
                    qs = slice(s * w, (s + 1) * w)
                    strip = lpool.tile([1, w], F32, tag="strip")
                    nc.vector.reciprocal(out=strip[:], in_=o_p[D:D + 1, qs])
                    lb = lpool.tile([D, w], F32, tag="lb")
                    nc.gpsimd.partition_broadcast(lb[:], strip[:])
                    nc.vector.tensor_mul(onorm[h * D:(h + 1) * D, qs], o_p[0:D, qs], lb[:])

            def stream2(qb, pair, onorm, splits, prek=None, prev_v=None):
                # both heads of the pair interleaved in one software-pipelined
                # sequence: each exp is covered by the other head's matmuls and
                # the exp chain never breaks at a head boundary
                q0 = qb * QB
                nk = (q0 + QB) // KB
                nfull = nk - 4
                ngrp = nk // 2
                o_ps_h = [o_ps.tile([D + 1, QB], F32, tag="o", name=f"o2_{h}")
                          for h in range(2)]

                def grp2(h, j2, diag):
                    bsl = slice(h * D, h * D + D)
                    hb = pair * 2 + h
                    vat = vslot(hb, 0)
                    o_p = o_ps_h[h]
                    qrhs = qt_sb[bsl, pair, q0:q0 + QB]
                    r0 = (j2 - nfull) * KB if diag else 0
                    r1 = r0 + KB
                    w1 = QB - r1

                    def ss():
                        sp = s_ps.tile([128, 2 * QB], F32, tag="s", name="sp")
                        pt = ppool.tile([128, 2 * QB], FP8, tag="p", name="pt")
                        if diag:
                            nc.tensor.matmul(sp[:, r0:QB], kdr(pair, j2, h),
                                             bcast2(qrhs[:, r0:]),
                                             start=True, stop=True, perf_mode=DR)
                            nc.tensor.matmul(sp[:, QB:QB + w1], kdr(pair, j2 + 1, h),
                                             bcast2(qrhs[:, r1:]),
                                             start=True, stop=True, perf_mode=DR)
                            mm((QB - r0 + w1) * PE_NS / 2)
                            nc.scalar.activation(out=pt[:, r0:QB + w1], in_=sp[:, r0:QB + w1],
                                                 func=EXP, scale=SCALE / (WSCALE * WSCALE))
                            clk["act"] = max(clk["act"], clk["pe"] + o["sem_lat"])                                 + (QB - r0 + w1) * ACT_NS + EXP_OVH
                        else:
                            for jj in range(2):
                                nc.tensor.matmul(sp[:, jj * QB:(jj + 1) * QB],
                                                 kdr(pair, j2 + jj, h), bcast2(qrhs),
                                                 start=True, stop=True, perf_mode=DR)
                            mm(2 * QB * PE_NS / 2)
                            nc.scalar.activation(out=pt[:], in_=sp[:], func=EXP,
                                                 scale=SCALE / (WSCALE * WSCALE))
                            clk["act"] = max(clk["act"], clk["pe"] + o["sem_lat"])                                 + 2 * QB * ACT_NS + EXP_OVH
                        return pt

                    def oo(pt):
                        if diag:
                            if not o["tri_in_ss"]:
                                tri_e = nc.gpsimd if o["tri_eng"] == "pool" else nc.vector
                                tri_e.tensor_tensor(out=pt[:, r0:r0 + KB],
                                                    in0=pt[:, r0:r0 + KB], in1=tri_sb[:], op=MUL)
                                tri_e.tensor_tensor(out=pt[:, QB:QB + KB],
                                                    in0=pt[:, QB:QB + KB], in1=tri_sb[:], op=MUL)
                            nc.tensor.matmul(o_p[:, r0:QB], vat(j2),
                                             bcast2p(pt[:, r0:QB]),
                                             start=(j2 == 0), stop=False, perf_mode=DR)
                            nc.tensor.matmul(o_p[:, r1:QB], vat(j2 + 1),
                                             bcast2p(pt[:, QB:QB + w1]),
                                             start=False, stop=(j2 + 1 == nk - 1),
                                             perf_mode=DR)
                            mm((QB - r0 + w1) * PE_NS / 2)
                        else:
                            for jj in range(2):
                                j = j2 + jj
                                nc.tensor.matmul(o_p[:], vat(j),
                                                 bcast2p(pt[:, jj * QB:(jj + 1) * QB]),
                                                 start=(j == 0), stop=False, perf_mode=DR)
                            mm(2 * QB * PE_NS / 2)
                    return ss, oo

                seq = []
                for g in range(ngrp):
                    j2 = 2 * g
                    diag = j2 >= nfull
                    for h in range(2):
                        seq.append((h, g, grp2(h, j2, diag)))
                prev = None
                for idx, (h, g, (ss, oo)) in enumerate(seq):
                    if idx == max(0, len(seq) - 4) and prek is not None:
                        prek()
                    pt = ss()
                    pace(clk["act"] + o["ret_lat"])
                    if prev is not None:
                        if prev[3] == max(0, len(seq) - 4) and prev_v is not None:
                            prev_v()
                        prev[1](prev[0])
                        if prev[4] == ngrp - 1:
                            normalize(o_ps_h[prev[2]], onorm, prev[2], splits)
                    prev = (pt, oo, h, idx, g)
                if prev[3] == max(0, len(seq) - 4) and prev_v is not None:
                    prev_v()
                prev[1](prev[0])
                normalize(o_ps_h[prev[2]], onorm, prev[2], splits)
                pace(clk["pe"] + o["end_fill"])

            def stream(qb, pair, h, onorm, splits, prek=None, prev_v=None,
                       mid=None):
                q0 = qb * QB
                nk = (q0 + QB) // KB
                nfull = nk - 4
                bsl = slice(h * D, h * D + D)
                hb = pair * 2 + h
                vat = vslot(hb, 0)
                o_p = o_ps.tile([D + 1, QB], F32, tag="o")
                qrhs = qt_sb[bsl, pair, q0:q0 + QB]

                def grp(j2, diag):
                    r0 = (j2 - nfull) * KB if diag else 0
                    r1 = r0 + KB
                    w1 = QB - r1

                    def ss():
                        sp = s_ps.tile([128, 2 * QB], F32, tag="s", name="sp")
                        pt = ppool.tile([128, 2 * QB], FP8, tag="p", name="pt")
                        if diag:
                            nc.tensor.matmul(sp[:, r0:QB], kdr(pair, j2, h),
                                             bcast2(qrhs[:, r0:]),
                                             start=True, stop=True, perf_mode=DR)
                            nc.tensor.matmul(sp[:, QB:QB + w1], kdr(pair, j2 + 1, h),
                                             bcast2(qrhs[:, r1:]),
                                             start=True, stop=True, perf_mode=DR)
                            mm((QB - r0 + w1) * PE_NS / 2)
                            nc.scalar.activation(out=pt[:, r0:QB + w1], in_=sp[:, r0:QB + w1],
                                                 func=EXP, scale=SCALE / (WSCALE * WSCALE))
                            clk["act"] = max(clk["act"], clk["pe"] + o["sem_lat"]) \
                                + (QB - r0 + w1) * ACT_NS + EXP_OVH
                            if o["tri_in_ss"]:
                                tri_e = nc.gpsimd if o["tri_eng"] == "pool" else nc.vector
                                tri_e.tensor_tensor(out=pt[:, r0:r0 + KB],
                                                    in0=pt[:, r0:r0 + KB], in1=tri_sb[:], op=MUL)
                                tri_e.tensor_tensor(out=pt[:, QB:QB + KB],
                                                    in0=pt[:, QB:QB + KB], in1=tri_sb[:], op=MUL)
                        else:
                            for jj in range(2):
                                j = j2 + jj
                                nc.tensor.matmul(sp[:, jj * QB:(jj + 1) * QB],
                                                 kdr(pair, j, h), bcast2(qrhs),
                                                 start=True, stop=True, perf_mode=DR)
                            mm(2 * QB * PE_NS / 2)
                            nc.scalar.activation(out=pt[:], in_=sp[:], func=EXP,
                                                 scale=SCALE / (WSCALE * WSCALE))
                            clk["act"] = max(clk["act"], clk["pe"] + o["sem_lat"]) \
                                + 2 * QB * ACT_NS + EXP_OVH
                        return pt

                    def oo(pt):
                        if diag:
                            if not o["tri_in_ss"]:
                                tri_e = nc.gpsimd if o["tri_eng"] == "pool" else nc.vector
                                tri_e.tensor_tensor(out=pt[:, r0:r0 + KB],
                                                    in0=pt[:, r0:r0 + KB], in1=tri_sb[:], op=MUL)
                                tri_e.tensor_tensor(out=pt[:, QB:QB + KB],
                                                    in0=pt[:, QB:QB + KB], in1=tri_sb[:], op=MUL)
                            nc.tensor.matmul(o_p[:, r0:QB], vat(j2),
                                             bcast2p(pt[:, r0:QB]),
                                             start=(j2 == 0), stop=False, perf_mode=DR)
                            nc.tensor.matmul(o_p[:, r1:QB], vat(j2 + 1),
                                             bcast2p(pt[:, QB:QB + w1]),
                                             start=False, stop=(j2 + 1 == nk - 1),
                                             perf_mode=DR)
                            mm((QB - r0 + w1) * PE_NS / 2)
                        else:
                            for jj in range(2):
                                j = j2 + jj
                                nc.tensor.matmul(o_p[:], vat(j),
                                                 bcast2p(pt[:, jj * QB:(jj + 1) * QB]),
                                                 start=(j == 0), stop=False, perf_mode=DR)
                            mm(2 * QB * PE_NS / 2)
                    return ss, oo

                groups = [grp(j2, False) for j2 in range(0, nfull, 2)] \
                    + [grp(j2, True) for j2 in range(nfull, nk, 2)]
                # software-pipelined: SS/exp of groups g+1..g+depth run before
                # OO of g so the next S matmuls (plus fillers) cover the exp
                # latency; the queue is GLOBAL, so a stream's last OO(s) and
                # its normalize are emitted after the NEXT stream's first S
                # matmuls -- the ACT queue never drains at a stream turn
                ndiag = len(groups) - 2  # first group whose SS reads this tch's K
                depth = o["pipe_depth"]
                for gi, (ss, oo) in enumerate(groups):
                    if gi == min(1, ndiag) and mid is not None:
                        mid()
                    if gi == ndiag and prek is not None:
                        prek()
                    pt = ss()
                    pace(clk["act"] + o["ret_lat"])
                    if len(pend) >= depth:
                        pop_oo()
                    hook = prev_v if gi == ndiag else None
                    after = None
                    if gi == len(groups) - 1:
                        def after(o_p=o_p, onorm=onorm, h=h, splits=splits):
                            normalize(o_p, onorm, h, splits)
                    pend.append((pt, oo, hook, after))
                if not o["carry_oo"]:
                    while pend:
                        pop_oo()
                pace(clk["pe"] + o["end_fill"])

            # ---- main schedule ----------------------------------------------
            nc.vector.memset(kt_sb[:, 0, :], 0.0)
            nc.vector.memset(kt_sb[:, 2 * NKB + 1, :], 0.0)
            if o["warmup"]:
                # climb the PE p-state ramp on junk zero matmuls while the
                # prologue DMAs land; results are discarded
                wu_ps = s_ps.tile([128, 2 * QB], F32, tag="s", name="wu")
                wu_lhs = kt_sb[0:64, 0::(2 * NKB + 1), :]
                wu_rhs = kt_sb[0:64, 0, :].unsqueeze(1).broadcast_to([64, 2, KB])
                for _ in range(o["warmup"]):
                    nc.tensor.matmul(wu_ps[:, 0:KB], wu_lhs, wu_rhs,
                                     start=True, stop=True, perf_mode=DR)
            emit_x_dma(0)
            vv = v_sb.rearrange("p b i (h w) -> p b i h w", w=VSLOT)
            nc.vector.memset(vv[:, :, 0, :, D:D + 1], 1.0)
            nc.vector.memset(vv[:, :, 1, :, D:D + 1], 0.0)

            def queue_proj(tch):
                qs = q_units(tch)
                q0 = [u for u in qs if u[0] == "qp0"]
                q1 = [u for u in qs if u[0] == "qp1"]
                k0, vs, k1 = kv_units(tch)
                for t, c, f, a in q0 + k0 + vs + q1 + k1:
                    fillers.append((f"{t}@{tch}", c, f, a))

            pending_y = []
            for qb in range(NQB):
                if qb == 0:
                    queue_proj(0)
                if 0 < qb < NQB - 1 and not o["x_upfront"]:
                    emit_x_dma(qb + 1)
                # this phase's Q (and for streams' diagonals, K/V) must be
                # emitted before the attention that reads it
                drain(lambda t: t == f"qp0@{qb}")
                if qb < NQB - 1:
                    queue_proj(qb + 1)
                onorms = [onpool.tile([128, QB], BF16, tag="on", name=f"on{qb}_{i}") for i in range(2)]
                splits = o["norm_splits_last"] if qb == NQB - 1 else o["norm_splits"]
                if o["merge_heads"]:
                    for si, pair in enumerate((0, 1)):
                        clk["allow_y"] = si < 1 or qb == NQB - 1
                        if pair == 1:
                            drain(lambda t: t == f"qp1@{qb}")
                            if o["early_qp0"] and qb < NQB - 1:
                                drain(lambda t: t == f"qp0@{qb + 1}")
                        prek = lambda p=pair: drain(lambda t: t == f"kp{p}@{qb}")
                        prev_v = (lambda: drain(
                            lambda t: t == f"v@{qb}")) if pair == 0 else None
                        stream2(qb, pair, onorms[pair], splits, prek=prek,
                                prev_v=prev_v)
                        if si == 0 and pending_y and (
                                qb == NQB - 1 or o["y_defer"] == 1):
                            fillers_y.extend(pending_y)
                            pending_y = []
                    pending_y = pending_y + y_units(qb, onorms)
                    continue
                si_order = ((0, 0), (1, 0), (0, 1), (1, 1)) if o["si_interleave"] \
                    else ((0, 0), (0, 1), (1, 0), (1, 1))
                for si, (pair, h) in enumerate(si_order):
                    clk["allow_y"] = si < o["y_si"] or qb == NQB - 1
                    if pair == 1 and h == 0:
                        drain(lambda t: t == f"qp1@{qb}")
                        if o["early_qp0"] and qb < NQB - 1 and not o["si_interleave"]:
                            drain(lambda t: t == f"qp0@{qb + 1}")
                    if o["si_interleave"] and si == 2 and o["early_qp0"] and qb < NQB - 1:
                        drain(lambda t: t == f"qp0@{qb + 1}")
                    prek = (lambda p=pair: drain(
                        lambda t: t == f"kp{p}@{qb}")) if h == 0 else None
                    prev_v = (lambda: drain(
                        lambda t: t == f"v@{qb}")) if (pair == 0 and h == 0) else None
                    def mid_fn(q=qb):
                        drain(lambda t: t == f"qp1@{q}")
                        if o["early_qp0"] and q < NQB - 1:
                            drain(lambda t: t == f"qp0@{q + 1}")
                    mid = mid_fn if (o["mid_qp1"] and si == 1) else None
                    stream(qb, pair, h, onorms[pair], splits, prek=prek, prev_v=prev_v,
                           mid=mid)
                    if si == 0 and pending_y and (
                            qb == NQB - 1 or o["y_defer"] == 1):
                        fillers_y.extend(pending_y)
                        pending_y = []
                pending_y = pending_y + y_units(qb, onorms)
            while pend:
                pop_oo()
            clk["allow_y"] = True
            while fillers or fillers_y:
                emit_from(fillers if fillers else fillers_y)
            for _, _, fn, _a in pending_y:
                fn()

    nc.compile()
    return nc


_NC = {}


def _get_program(opts=None):
    key = tuple(sorted((opts or {}).items()))
    if key not in _NC:
        _NC[key] = build_program(opts)
    return _NC[key]


def _make_in_maps(x, Wq, Wk, Wv, Wp):
    bf = ml_dtypes.bfloat16
    f8 = ml_dtypes.float8_e4m3
    x32 = np.asarray(x, np.float32)
    xh = x32.astype(f8)
    xl = (x32 - xh.astype(np.float32)).astype(f8)

    def wsplit(W):
        wt = np.asarray(W, np.float32).T * WSCALE
        hi = wt.astype(f8)
        lo = (wt - hi.astype(np.float32)).astype(f8)
        return hi, lo

    wqh, wql = wsplit(Wq)
    wkh, wkl = wsplit(Wk)
    wvh, wvl = wsplit(Wv)
    wpt = np.ascontiguousarray((np.asarray(Wp, np.float32).T / WSCALE).astype(bf))
    tri = (np.arange(KB)[:, None] <= np.arange(KB)[None, :]).astype(f8)
    in_maps = []
    for c in range(N_CORES):
        b, hg = c // 4, c % 4
        gsl = slice(hg * GE, (hg + 1) * GE)
        in_maps.append({
            "xh": np.ascontiguousarray(xh[b].T),
            "xl": np.ascontiguousarray(xl[b].T),
            "wqh": np.ascontiguousarray(wqh[:, gsl]),
            "wql": np.ascontiguousarray(wql[:, gsl]),
            "wkh": np.ascontiguousarray(wkh[:, gsl]),
            "wkl": np.ascontiguousarray(wkl[:, gsl]),
            "wvh": np.ascontiguousarray(wvh[:, gsl]),
            "wvl": np.ascontiguousarray(wvl[:, gsl]),
            "wpt": np.ascontiguousarray(wpt[gsl, :]),
            "tri": tri,
        })
    return in_maps


def run_cores(x, Wq, Wk, Wv, Wp, bp, **spmd_kwargs):
    """Run the 8-core program; returns (y_full, BassKernelResults)."""
    nc = _get_program()
    in_maps = _make_in_maps(x, Wq, Wk, Wv, Wp)
    res = run_bass_kernel_spmd(nc, in_maps, list(range(N_CORES)), **spmd_kwargs)
    parts = [res.results[c]["y"] for c in range(N_CORES)]
    y = np.empty((B, T, E), np.float32)
    for b in range(B):
        acc = parts[4 * b].astype(np.float32)
        for hg in range(1, 4):
            acc = acc + parts[4 * b + hg].astype(np.float32)
        y[b] = acc
    y += np.asarray(bp, np.float32)[None, None, :]
    return y, res


def kernel(x, Wq, Wk, Wv, Wp, bp):
    y, _ = run_cores(x, Wq, Wk, Wv, Wp, bp)
    return y


def bench(x, Wq, Wk, Wv, Wp, bp, iters=12):
    """Time repeated on-device executions of the compiled program."""
    import time

    import jax
    import numpy as np_
    from jax.experimental.shard_map import shard_map
    from jax.sharding import Mesh, NamedSharding, PartitionSpec

    from concourse import bass2jax, mybir as mb

    nc = _get_program()
    in_maps = _make_in_maps(x, Wq, Wk, Wv, Wp)
    n_cores = N_CORES
    bass2jax.install_neuronx_cc_hook()

    partition_name = nc.partition_id_tensor.name if nc.partition_id_tensor else None
    in_names, out_names, out_avals, zero_outs = [], [], [], []
    for alloc in nc.m.functions[0].allocations:
        if not isinstance(alloc, mb.MemoryLocationSet):
            continue
        name = alloc.memorylocations[0].name
        if alloc.kind == "ExternalInput":
            if name != partition_name:
                in_names.append(name)
        elif alloc.kind == "ExternalOutput":
            out_names.append(name)
            shape = tuple(alloc.tensor_shape)
            dtype = mb.dt.np(alloc.dtype)
            out_avals.append(jax.core.ShapedArray(shape, dtype))
            zero_outs.append(np_.zeros(shape, dtype))
    n_params = len(in_names)
    all_in_names = in_names + out_names
    if partition_name is not None:
        all_in_names = all_in_names + [partition_name]

    def _body(*args):
        operands = list(args)
        if partition_name is not None:
            operands.append(bass2jax.partition_id_tensor())
        outs = bass2jax._bass_exec_p.bind(
            *operands,
            out_avals=tuple(out_avals),
            in_names=tuple(all_in_names),
            out_names=tuple(out_names),
            lowering_input_output_aliases=(),
            sim_require_finite=True,
            sim_require_nnan=True,
            nc=nc,
        )
        return tuple(outs)

    devices = jax.devices()[:n_cores]
    mesh = Mesh(np_.asarray(devices), ("core",))
    donate = tuple(range(n_params, n_params + len(out_names)))
    sharded = jax.jit(
        shard_map(_body, mesh=mesh,
                  in_specs=(PartitionSpec("core"),) * (n_params + len(out_names)),
                  out_specs=(PartitionSpec("core"),) * len(out_names),
                  check_rep=False),
        donate_argnums=donate, keep_unused=True,
    )
    sh = NamedSharding(mesh, PartitionSpec("core"))
    concat_in = [
        jax.device_put(
            np_.concatenate([np_.asarray(in_maps[c][nm]) for c in range(n_cores)], axis=0), sh)
        for nm in in_names
    ]
    zeros_np = [np_.zeros((n_cores * z.shape[0], *z.shape[1:]), z.dtype) for z in zero_outs]

    times = []
    out_arrs = None
    for it in range(iters):
        dz = [jax.device_put(z, sh) for z in zeros_np]
        jax.block_until_ready(dz)
        t0 = time.perf_counter()
        out_arrs = sharded(*concat_in, *dz)
        jax.block_until_ready(out_arrs)
        times.append(time.perf_counter() - t0)

    parts = [
        np_.asarray(out_arrs[i]).reshape(n_cores, *out_avals[i].shape)
        for i, nm in enumerate(out_names)
    ]
    yi = out_names.index("y")
    y = np_.empty((B, T, E), np_.float32)
    for b in range(B):
        acc = parts[yi][4 * b].astype(np_.float32)
        for hg in range(1, 4):
            acc = acc + parts[yi][4 * b + hg].astype(np_.float32)
        y[b] = acc
    y += np_.asarray(bp, np_.float32)[None, None, :]
    return y, times
